# revision 7
# baseline (speedup 1.0000x reference)
"""Trainium2 Bass kernel for a GPT-2-style transformer block.

B=4, T=1024, C=768, H=12 heads (HD=64). 8 NeuronCores.

Sharding: 2 cores per batch sequence. Each core is fed a block-permuted
copy of its sequence (own query blocks at even block positions), computes
K/V for the full sequence locally (no collectives), runs causal attention
for its 512 query tokens with data-driven masks, and the full MLP for
those tokens. Host re-assembles the [B,T,C] output.

QKV/Wo/Wfc/Wproj matmuls run in bf16; residual/LN paths stay f32.
"""

import numpy as np
import ml_dtypes

P = 128
B, T, C, H = 4, 1024, 768, 12
HD = C // H        # 64
CJ = C // P        # 6 C-chunks
NT = T // P        # 8 token tiles
TQ = 512           # own query tokens per core
NQT = TQ // P      # 4 q slots
FC = 4 * C         # 3072
FCJ = FC // P      # 24
GELU_C = 0.035677408136300125  # sqrt(2/pi)*0.044715 -> tanh(c*x^4)
N_CORES = 8

_CACHED = {}


def _build_nc():
    import concourse.bass as bass
    from concourse import bacc, mybir
    import concourse.tile as tile
    from concourse.masks import make_identity
    from contextlib import ExitStack

    F32 = mybir.dt.float32
    F32R = mybir.dt.float32r
    BF16 = mybir.dt.bfloat16
    AF = mybir.ActivationFunctionType
    ALU = mybir.AluOpType

    nc = bacc.Bacc()

    xp_d = nc.declare_dram_parameter("xp", [T, C], F32, isOutput=False)
    qg_d = nc.declare_dram_parameter("qg", [TQ], F32, isOutput=False)
    kg_d = nc.declare_dram_parameter("kg", [P, NT], F32, isOutput=False)
    wqkv_d = nc.declare_dram_parameter("wqkv", [C, 3 * C], BF16, isOutput=False)
    bqkv_d = nc.declare_dram_parameter("bqkv", [P, 18], F32, isOutput=False)
    bv_d = nc.declare_dram_parameter("bv", [C], F32, isOutput=False)
    wo_d = nc.declare_dram_parameter("wo", [C, C], BF16, isOutput=False)
    bo_d = nc.declare_dram_parameter("bo", [P, CJ], F32, isOutput=False)
    wfc_d = nc.declare_dram_parameter("wfc", [C, FC], BF16, isOutput=False)
    bfc_d = nc.declare_dram_parameter("bfc", [P, FCJ], F32, isOutput=False)
    wproj_d = nc.declare_dram_parameter("wproj", [FC, C], BF16, isOutput=False)
    bproj_d = nc.declare_dram_parameter("bproj", [P, CJ], F32, isOutput=False)
    out_d = nc.declare_dram_parameter("out", [TQ, C], F32, isOutput=True)

    def bcast_dma(engine, dst, dram_handle, offset, n):
        """DMA [n] DRAM vector broadcast across 128 partitions -> dst[128, n]."""
        ap = dram_handle[:]
        src = bass.AP(tensor=ap.tensor, offset=offset, ap=[[0, P], [1, n]])
        engine.dma_start(dst, src)

    with tile.TileContext(nc) as tc, ExitStack() as ctx:
        persist = ctx.enter_context(tc.tile_pool(name="persist", bufs=1))
        work = ctx.enter_context(tc.tile_pool(name="work", bufs=3))

        # ---------- constants ----------
        ident = persist.tile([P, P], F32, tag="ident")
        make_identity(nc, ident)
        ident_r = persist.tile([P, P], F32R, tag="identr")
        nc.vector.tensor_copy(ident_r, ident)
        eps_t = persist.tile([P, 1], F32, tag="eps")
        nc.vector.memset(eps_t, 1e-5)
        ones1_f32 = persist.tile([1, HD], F32, tag="ones1f")
        nc.vector.memset(ones1_f32, 1.0)
        ones1 = persist.tile([1, HD], BF16, tag="ones1")
        nc.vector.tensor_copy(ones1, ones1_f32)
        ones_col_f32 = persist.tile([P, 1], F32, tag="onescol")
        nc.vector.memset(ones_col_f32, 1.0)
        ones_col_r = persist.tile([P, 1], F32R, tag="onescolr")
        nc.vector.tensor_copy(ones_col_r, ones_col_f32)
        ones_row_bf = persist.tile([1, P], BF16, tag="onesrow")
        nc.vector.tensor_copy(ones_row_bf, ones_col_f32[0:1, 0:1].to_broadcast([1, P]))

        # ---------- x DMAs first (critical path), V weight pieces early ----------
        x_own = [persist.tile([P, C], F32, tag=f"xo{t}", name=f"xo{t}")
                 for t in range(NQT)]
        x1T = [persist.tile([P, TQ], F32R, tag=f"x1T{m}", name=f"x1T{m}")
               for m in range(CJ)]

        wq3 = wqkv_d[:, :].rearrange("(o p) n -> p o n", p=P)

        # small bias loads on gpsimd queue (host pre-transposed, contiguous)
        bqkv_po = persist.tile([P, 18], F32, tag="bqkv")
        nc.gpsimd.dma_start(bqkv_po, bqkv_d[:, :])
        bv_b = persist.tile([P, C], F32, tag="bv")
        bcast_dma(nc.gpsimd, bv_b, bv_d, 0, C)

        mask_cols = {0: (0, 128), 1: (0, 128), 2: (0, 256), 3: (0, 256),
                     4: (256, 128), 5: (256, 128), 6: (256, 256), 7: (256, 256)}
        masks = {}

        with tc.tile_pool(name="attn_live", bufs=1) as attn_live:
            # packed head-pair K/Q tiles: partitions [0:64]=head 2j, [64:128]=head 2j+1
            kTp = [attn_live.tile([P, T], BF16, tag=f"kTp{j}", name=f"kTp{j}")
                   for j in range(CJ)]
            qTp = [attn_live.tile([P, TQ], BF16, tag=f"qTp{j}", name=f"qTp{j}")
                   for j in range(CJ)]
            v_aug = [attn_live.tile([P, H, HD + 1], BF16, tag=f"vaug{t}",
                                    name=f"vaug{t}") for t in range(NT)]
            yT = [attn_live.tile([P, TQ], BF16, tag=f"yT{j}", name=f"yT{j}")
                  for j in range(CJ)]

            with tc.tile_pool(name="ph12", bufs=1) as ph12, \
                 tc.tile_pool(name="wstream", bufs=1) as wstream, \
                 tc.tile_pool(name="att", bufs=3) as att, \
                 tc.tile_pool(name="ps_tr1", bufs=2, space="PSUM") as ps_tr, \
                 tc.tile_pool(name="ps_mm", bufs=3, space="PSUM") as ps_mm, \
                 tc.tile_pool(name="ps_av", bufs=2, space="PSUM") as ps_av, \
                 tc.tile_pool(name="ps_bc", bufs=1, space="PSUM") as ps_bc:

                # x tiles: own (even permuted positions) persist; odd in ph12
                x_sb = []
                for t in range(NT):
                    if t % 2 == 0:
                        x_sb.append(x_own[t // 2])
                    else:
                        xt = ph12.tile([P, C], F32, tag=f"x{t}", name=f"x{t}")
                        x_sb.append(xt)
                nc.sync.dma_start(x_sb[0], xp_d[0:P, :])
                nc.sync.dma_start(x_sb[1], xp_d[P:2 * P, :])
                # V weight pieces (both halves upfront, sync queue)
                vw = [wstream.tile([P, CJ, 384], BF16, tag=f"vw{h}", name=f"vw{h}")
                      for h in range(2)]
                for half in range(2):
                    nc.sync.dma_start(
                        vw[half],
                        wq3[:, :, 2 * C + half * 384: 2 * C + (half + 1) * 384])
                for t in range(2, NT):
                    nc.sync.dma_start(x_sb[t], xp_d[t * P:(t + 1) * P, :])
                # K/Q weight pieces right behind on sync queue
                wtk = [wstream.tile([P, CJ, 384], BF16, tag=f"wtk{k}",
                                    name=f"wtk{k}") for k in range(2)]
                wtq = [wstream.tile([P, CJ, 384], BF16, tag=f"wtq{k}",
                                    name=f"wtq{k}") for k in range(2)]
                for kp in range(2):
                    nc.sync.dma_start(
                        wtk[kp], wq3[:, :, C + kp * 384: C + (kp + 1) * 384])
                    nc.sync.dma_start(
                        wtq[kp], wq3[:, :, kp * 384:(kp + 1) * 384])

                xlnT = [ph12.tile([P, T], BF16, tag=f"xlnT{j}", name=f"xlnT{j}")
                        for j in range(CJ)]

                # mask input loads (gpsimd queue, after bias loads)
                kg_po = ph12.tile([P, NT], F32, tag="kg")
                nc.gpsimd.dma_start(kg_po, kg_d[:, :])
                qg_b = ph12.tile([P, TQ], F32, tag="qgb")
                bcast_dma(nc.gpsimd, qg_b, qg_d, 0, TQ)

                # ---- LN1 + transpose + V projection, pipelined per tile ----
                for t in range(NT):
                    stats = work.tile([P, 3, 6], F32, tag="bnstats")
                    for g in range(3):
                        nc.vector.bn_stats(stats[:, g, :],
                                           x_sb[t][:, g * 256:(g + 1) * 256])
                    mv = work.tile([P, 2], F32, tag="bnmv")
                    nc.vector.bn_aggr(mv, stats)
                    rstd = work.tile([P, 1], F32, tag="rstd")
                    nc.scalar.activation(rstd, mv[:, 1:2], AF.Sqrt, bias=eps_t)
                    nc.vector.reciprocal(rstd, rstd)
                    nmr = work.tile([P, 1], F32, tag="nmr")
                    nc.vector.tensor_tensor(nmr, mv[:, 0:1], rstd, ALU.mult)
                    nc.vector.tensor_scalar(nmr, nmr, -1.0, None, ALU.mult)
                    xln = work.tile([P, C], F32R, tag="xln")
                    nc.scalar.activation(xln, x_sb[t], AF.Identity,
                                         bias=nmr, scale=rstd)
                    for j in range(CJ):
                        ptr = ps_tr.tile([P, P], F32R, tag="tr")
                        nc.tensor.transpose(ptr, xln[:, j * P:(j + 1) * P],
                                            ident_r)
                        dst = xlnT[j][:, t * P:(t + 1) * P]
                        if j % 2 == 0:
                            nc.vector.tensor_copy(dst, ptr)
                        else:
                            nc.scalar.copy(dst, ptr)
                    # V for this token tile: [128 tok, 384] x2, kc-outer to
                    # reuse the stationary xlnT chunk across halves
                    pmv = [ps_mm.tile([P, 512], F32, tag="mm", name=f"pmv{_h}")
                           for _h in range(2)]
                    for kc in range(CJ):
                        for half in range(2):
                            nc.tensor.matmul(
                                pmv[half][:, 0:384],
                                xlnT[kc][:, t * P:(t + 1) * P],
                                vw[half][:, kc, :],
                                start=(kc == 0), stop=(kc == CJ - 1),
                                skip_group_check=True)
                    for half in range(2):
                        nc.vector.tensor_tensor(
                            v_aug[t][:, half * 6:(half + 1) * 6, 0:HD],
                            pmv[half][:, 0:384].rearrange("p (h d) -> p h d", d=HD),
                            bv_b[:, half * 384:(half + 1) * 384].rearrange(
                                "p (h d) -> p h d", d=HD),
                            ALU.add)

                # masks + v_aug ones column (vector; before first AV use)
                for kc in range(NT):
                    off, w = mask_cols[kc]
                    m = persist.tile([P, w], BF16, tag=f"mask{kc}",
                                     name=f"mask{kc}")
                    nc.vector.tensor_scalar(
                        m, qg_b[:, off:off + w], kg_po[:, kc:kc + 1], None,
                        ALU.is_ge)
                    masks[kc] = m
                for t in range(NT):
                    nc.vector.tensor_copy(
                        v_aug[t][:, :, HD:HD + 1],
                        ones_col_f32.to_broadcast([P, H, 1]))

                # ---- K, Q, attention per feature tile j ----
                for kp in range(2):
                    for jl in range(3):
                        j = 3 * kp + jl
                        # K: both halves, kc-outer for stationary reuse
                        pmk = [ps_mm.tile([P, 512], F32, tag="mm",
                                          name=f"pmk{_h}")
                               for _h in range(2)]
                        for kc in range(CJ):
                            for half in range(2):
                                nc.tensor.matmul(
                                    pmk[half],
                                    wtk[kp][:, kc, jl * P:(jl + 1) * P],
                                    xlnT[kc][:, half * 512:(half + 1) * 512],
                                    start=(kc == 0), stop=(kc == CJ - 1),
                                    skip_group_check=True)
                        for half in range(2):
                            nc.vector.tensor_scalar(
                                kTp[j][:, half * 512:(half + 1) * 512],
                                pmk[half], bqkv_po[:, CJ + j:CJ + j + 1], None,
                                ALU.add)
                        # Q: own (even) blocks only
                        pmq = ps_mm.tile([P, 512], F32, tag="mm")
                        for kc in range(CJ):
                            own = xlnT[kc].rearrange(
                                "p (b c) -> p b c", c=P)[:, 0::2, :]
                            nc.tensor.matmul(
                                pmq, wtq[kp][:, kc, jl * P:(jl + 1) * P], own,
                                start=(kc == 0), stop=(kc == CJ - 1))
                        nc.vector.tensor_scalar(
                            qTp[j], pmq, bqkv_po[:, j:j + 1], None, ALU.add)

                        # ---- attention for the two heads of tile j ----
                        for hh in range(2):
                            h = 2 * j + hh
                            hs = slice(hh * HD, (hh + 1) * HD)
                            av = ps_av.tile([HD + 1, 512], F32, tag="av")
                            for kc in range(NT):
                                n0 = 0 if kc < 4 else 256
                                w = 512 - n0
                                sc = ps_mm.tile([P, 512], F32, tag="mm")
                                nc.tensor.matmul(
                                    sc[:, 0:w],
                                    kTp[j][hs, kc * P:(kc + 1) * P],
                                    qTp[j][hs, n0:512],
                                    start=True, stop=True)
                                ex = att.tile([P, 512], BF16, tag="exp")
                                nc.scalar.activation(ex[:, 0:w], sc[:, 0:w],
                                                     AF.Exp, scale=0.125)
                                off, wm = mask_cols[kc]
                                loc = off - n0
                                nc.vector.tensor_tensor(
                                    ex[:, loc:loc + wm], ex[:, loc:loc + wm],
                                    masks[kc], ALU.mult)
                                nc.tensor.matmul(
                                    av[:, n0:512], v_aug[kc][:, h, :],
                                    ex[:, 0:w],
                                    start=(kc == 0), stop=(kc == NT - 1),
                                    skip_group_check=True)
                            sums_bf = att.tile([1, 512], BF16, tag="sums")
                            nc.vector.tensor_copy(sums_bf, av[HD:HD + 1, :])
                            bc = ps_bc.tile([HD, 512], F32, tag="bc")
                            nc.tensor.matmul(bc, ones1, sums_bf,
                                             start=True, stop=True)
                            rb = att.tile([HD, 512], F32, tag="rb")
                            with nc.allow_low_precision(reason="softmax denom"):
                                nc.vector.reciprocal_approx_fast(rb, bc)
                            nc.vector.tensor_tensor(
                                yT[j][hs, :], av[0:HD, :], rb, ALU.mult)

            # ---- phase 4: x_own^T (transpose-accumulate) + Wo -> x1T ----
            with tc.tile_pool(name="wo_p", bufs=1) as wo_p:
                wo_t = wo_p.tile([P, CJ, C], BF16, tag="wo")
                nc.sync.dma_start(
                    wo_t, wo_d[:, :].rearrange("(o p) n -> p o n", p=P))
                bo_po = persist.tile([P, CJ], F32, tag="bo")
                nc.gpsimd.dma_start(bo_po, bo_d[:, :])
                with tc.tile_pool(name="ps_mm4", bufs=2, space="PSUM") as ps_mm4:
                    for m in range(CJ):
                        pm = ps_mm4.tile([P, TQ], F32, tag="mm")
                        for t in range(NQT):
                            nc.tensor.matmul(
                                pm[:, t * P:(t + 1) * P],
                                x_own[t][:, m * P:(m + 1) * P], ident,
                                is_transpose=True,
                                start=(t == 0), stop=False,
                                skip_group_check=True)
                        for kc in range(CJ):
                            nc.tensor.matmul(
                                pm, wo_t[:, kc, m * P:(m + 1) * P], yT[kc],
                                start=False, stop=(kc == CJ - 1),
                                skip_group_check=True)
                        with nc.allow_low_precision(reason="residual f32r"):
                            nc.vector.tensor_scalar(
                                x1T[m], pm, bo_po[:, m:m + 1], None, ALU.add)

        # ---------- phases 5-7: LN2 (feature-major), FC+gelu, proj+out ----------
        with tc.tile_pool(name="mlp_live", bufs=1) as mlp_live, \
             tc.tile_pool(name="ln2c_p", bufs=1) as ln2c_p:
            h1T = [mlp_live.tile([P, TQ], BF16, tag=f"h1T{m}", name=f"h1T{m}")
                   for m in range(FCJ)]
            bfc_po = persist.tile([P, FCJ], F32, tag="bfc")
            nc.gpsimd.dma_start(bfc_po, bfc_d[:, :])
            bproj_po = persist.tile([P, CJ], F32, tag="bproj")
            nc.gpsimd.dma_start(bproj_po, bproj_d[:, :])

            with tc.tile_pool(name="ph5", bufs=3) as ph5, \
                 tc.tile_pool(name="ps_st", bufs=1, space="PSUM") as ps_st, \
                 tc.tile_pool(name="ps_bc2", bufs=1, space="PSUM") as ps_bc2:
                # token stats via ones-column matmuls (partition reduction)
                mu_ps = ps_st.tile([1, TQ], F32, tag="mups", name="mups")
                sq_ps = ps_st.tile([1, TQ], F32, tag="sqps", name="sqps")
                for m in range(CJ):
                    nc.tensor.matmul(mu_ps, ones_col_r, x1T[m],
                                     start=(m == 0), stop=(m == CJ - 1))
                for m in range(CJ):
                    sq = ph5.tile([P, TQ], F32R, tag="sq")
                    if m % 2 == 0:
                        nc.scalar.activation(sq, x1T[m], AF.Square)
                    else:
                        nc.gpsimd.tensor_tensor(sq, x1T[m], x1T[m], ALU.mult)
                    nc.tensor.matmul(sq_ps, ones_col_r, sq,
                                     start=(m == 0), stop=(m == CJ - 1))
                mu_f = ln2c_p.tile([1, TQ], F32, tag="muf")
                nc.vector.tensor_scalar(mu_f, mu_ps, 1.0 / C, None, ALU.mult)
                var_f = ln2c_p.tile([1, TQ], F32, tag="varf")
                nc.vector.tensor_scalar(var_f, sq_ps, 1.0 / C, None, ALU.mult)
                musq = ln2c_p.tile([1, TQ], F32, tag="musq")
                nc.vector.tensor_tensor(musq, mu_f, mu_f, ALU.mult)
                nc.vector.tensor_tensor(var_f, var_f, musq, ALU.subtract)
                rstd_f = ln2c_p.tile([1, TQ], F32, tag="rstdf")
                nc.scalar.activation(rstd_f, var_f, AF.Sqrt, bias=eps_t[0:1, :])
                nc.vector.reciprocal(rstd_f, rstd_f)
                murstd_f = ln2c_p.tile([1, TQ], F32, tag="murstdf")
                nc.vector.tensor_tensor(murstd_f, mu_f, rstd_f, ALU.mult)
                rstd_bf = ln2c_p.tile([1, TQ], BF16, tag="rstdbf")
                nc.vector.tensor_copy(rstd_bf, rstd_f)
                murstd_bf = ln2c_p.tile([1, TQ], BF16, tag="murstdbf")
                nc.vector.tensor_copy(murstd_bf, murstd_f)
                # broadcast across partitions via K=1 matmuls
                rstd_bc_ps = ps_bc2.tile([P, TQ], F32, tag="rstdbc",
                                         name="rstdbc")
                nc.tensor.matmul(rstd_bc_ps, ones_row_bf, rstd_bf,
                                 start=True, stop=True)
                murstd_bc_ps = ps_bc2.tile([P, TQ], F32, tag="murstdbc",
                                           name="murstdbc")
                nc.tensor.matmul(murstd_bc_ps, ones_row_bf, murstd_bf,
                                 start=True, stop=True)
                rstd_bc = ln2c_p.tile([P, TQ], F32, tag="rstdbcs")
                nc.vector.tensor_copy(rstd_bc, rstd_bc_ps)
                murstd_bc = ln2c_p.tile([P, TQ], F32, tag="murstdbcs")
                nc.vector.tensor_copy(murstd_bc, murstd_bc_ps)
                # pre-normalized bf16 x1 for the FC matmul
                x1nb = [ln2c_p.tile([P, TQ], BF16, tag=f"x1nb{m}",
                                    name=f"x1nb{m}") for m in range(CJ)]
                for m in range(CJ):
                    tmp = ph5.tile([P, TQ], F32, tag="x1s")
                    nc.vector.tensor_tensor(tmp, x1T[m], rstd_bc, ALU.mult)
                    nc.gpsimd.tensor_tensor(x1nb[m], tmp, murstd_bc,
                                            ALU.subtract)

            # ---------- phase 6: FC + gelu ----------
            with tc.tile_pool(name="wfc_p", bufs=3) as wfc_p, \
                 tc.tile_pool(name="ph6", bufs=3) as ph6, \
                 tc.tile_pool(name="ps_mm6", bufs=2, space="PSUM") as ps_mm6:
                wfc3 = wfc_d[:, :].rearrange("(o p) n -> p o n", p=P)
                for m in range(FCJ):
                    if m % 4 == 0:
                        wt4 = wfc_p.tile([P, CJ, 512], BF16, tag="wfc")
                        nc.sync.dma_start(
                            wt4, wfc3[:, :, m * P:(m + 4) * P])
                    ml = m % 4
                    pm = ps_mm6.tile([P, TQ], F32, tag="mm")
                    for kc in range(CJ):
                        nc.tensor.matmul(pm, wt4[:, kc, ml * P:(ml + 1) * P],
                                         x1nb[kc],
                                         start=(kc == 0), stop=(kc == CJ - 1),
                                         skip_group_check=True)
                    xb = ph6.tile([P, TQ], F32, tag="xb")
                    nc.vector.tensor_scalar(xb, pm, bfc_po[:, m:m + 1], None,
                                            ALU.add)
                    q4 = ph6.tile([P, TQ], F32, tag="q4")
                    nc.gpsimd.tensor_tensor(q4, xb, xb, ALU.mult)
                    nc.gpsimd.tensor_tensor(q4, q4, q4, ALU.mult)
                    u = ph6.tile([P, TQ], F32, tag="u")
                    nc.scalar.activation(u, q4, AF.Tanh, scale=GELU_C)
                    nc.vector.tensor_scalar(u, u, 0.5, 0.5, ALU.mult, ALU.add)
                    nc.vector.tensor_tensor(h1T[m], xb, u, ALU.mult)

            # ---------- phase 7: proj + residual -> out ----------
            with tc.tile_pool(name="wpj_p", bufs=4) as wpj_p, \
                 tc.tile_pool(name="ph7", bufs=2) as ph7, \
                 tc.tile_pool(name="out_p", bufs=1) as out_p, \
                 tc.tile_pool(name="ps_pj", bufs=1, space="PSUM") as ps_pj:
                pms = [ps_pj.tile([P, TQ], F32, tag=f"pj{m}", name=f"pj{m}")
                       for m in range(CJ)]
                wpj3 = wproj_d[:, :].rearrange("(o p) n -> p o n", p=P)
                for kc in range(FCJ):
                    if kc % 2 == 0:
                        wt2 = wpj_p.tile([P, 2, C], BF16, tag="wpj")
                        nc.sync.dma_start(
                            wt2, wpj3[:, kc:kc + 2, :])
                    kl = kc % 2
                    for m in range(CJ):
                        nc.tensor.matmul(
                            pms[m], wt2[:, kl, m * P:(m + 1) * P], h1T[kc],
                            start=(kc == 0), stop=(kc == FCJ - 1))
                ojT = [ph7.tile([P, TQ], F32R, tag=f"ojT{m}", name=f"ojT{m}")
                       for m in range(CJ)]
                for m in range(CJ):
                    nc.vector.tensor_scalar(
                        ojT[m], pms[m], bproj_po[:, m:m + 1], None, ALU.add)
                    nc.vector.tensor_tensor(ojT[m], ojT[m], x1T[m], ALU.add)
                out_sb = [out_p.tile([P, C], F32, tag=f"osb{t}", name=f"osb{t}")
                          for t in range(NQT)]
                with tc.tile_pool(name="ps_tr7", bufs=2, space="PSUM") as ps_tr7:
                    for t in range(NQT):
                        for m in range(CJ):
                            ptr = ps_tr7.tile([P, P], F32R, tag="tr")
                            nc.tensor.transpose(
                                ptr, ojT[m][:, t * P:(t + 1) * P], ident_r)
                            dst = out_sb[t][:, m * P:(m + 1) * P]
                            if m % 2 == 0:
                                nc.vector.tensor_copy(dst, ptr)
                            else:
                                nc.scalar.copy(dst, ptr)
                        nc.sync.dma_start(out_d[t * P:(t + 1) * P, :],
                                          out_sb[t])

    nc.compile()
    return nc


def _get_nc():
    if "nc" not in _CACHED:
        _CACHED["nc"] = _build_nc()
    return _CACHED["nc"]


def _perm_blocks(p):
    return [p, 1 - p, 2 + p, 3 - p, 4 + p, 5 - p, 6 + p, 7 - p]


def _prepare(x, ln1_scale, ln1_bias, Wqkv, bqkv, Wo, bo,
             ln2_scale, ln2_bias, Wfc, bfc, Wproj, bproj):
    """Host-side prep: fold LN params into weights, permute qkv to
    [Q|K|V] layout, pre-transpose bias vectors, build per-core in_maps."""
    x = np.asarray(x, np.float32)
    Wqkv64 = np.asarray(Wqkv, np.float64)
    Wqkv64 = np.asarray(ln1_scale, np.float64)[:, None] * Wqkv64
    bqkv64 = np.asarray(bqkv, np.float64) + np.asarray(ln1_bias, np.float64) @ Wqkv64
    Wfc64 = np.asarray(Wfc, np.float64)
    Wfc64 = np.asarray(ln2_scale, np.float64)[:, None] * Wfc64
    bfc64 = np.asarray(bfc, np.float64) + np.asarray(ln2_bias, np.float64) @ Wfc64
    # Reference splits qkv per head: columns are [h0: q|k|v, h1: q|k|v, ...].
    colmap = np.arange(3 * C).reshape(H, 3, HD)
    qkv_perm = np.concatenate(
        [colmap[:, 0, :].ravel(), colmap[:, 1, :].ravel(), colmap[:, 2, :].ravel()])
    Wqkvp = Wqkv64.astype(np.float32)[:, qkv_perm]
    bqkvp = bqkv64.astype(np.float32)[qkv_perm]

    def po(v, cols):
        return np.ascontiguousarray(
            np.asarray(v, np.float32).reshape(cols, P).T)

    shared = {
        "wqkv": np.ascontiguousarray(Wqkvp.astype(ml_dtypes.bfloat16)),
        "bqkv": po(bqkvp, 18),
        "bv": np.ascontiguousarray(bqkvp[2 * C:]),
        "wo": np.ascontiguousarray(np.asarray(Wo, np.float32).astype(ml_dtypes.bfloat16)),
        "bo": po(bo, CJ),
        "wfc": np.ascontiguousarray(Wfc64.astype(ml_dtypes.bfloat16)),
        "bfc": po(bfc64.astype(np.float32), FCJ),
        "wproj": np.ascontiguousarray(np.asarray(Wproj, np.float32).astype(ml_dtypes.bfloat16)),
        "bproj": po(bproj, CJ),
    }
    in_maps = []
    own_toks = []
    for c in range(N_CORES):
        s, p = divmod(c, 2)
        blocks = _perm_blocks(p)
        tok = np.concatenate([np.arange(b * P, (b + 1) * P) for b in blocks])
        own = np.concatenate([np.arange(b * P, (b + 1) * P) for b in blocks[0::2]])
        own_toks.append((s, own))
        in_maps.append({
            "xp": np.ascontiguousarray(x[s][tok]),
            "qg": own.astype(np.float32),
            "kg": po(tok.astype(np.float32), NT),
            **shared,
        })
    return in_maps, own_toks


def kernel(x, ln1_scale, ln1_bias, Wqkv, bqkv, Wo, bo,
           ln2_scale, ln2_bias, Wfc, bfc, Wproj, bproj):
    from concourse.bass_utils import run_bass_kernel_spmd

    in_maps, own_toks = _prepare(x, ln1_scale, ln1_bias, Wqkv, bqkv, Wo, bo,
                                 ln2_scale, ln2_bias, Wfc, bfc, Wproj, bproj)
    nc = _get_nc()
    res = run_bass_kernel_spmd(nc, in_maps, list(range(N_CORES)))

    out = np.empty((B, T, C), np.float32)
    for c in range(N_CORES):
        s, own = own_toks[c]
        out[s][own] = res.results[c]["out"]
    return out


# revision 11
# speedup vs baseline: 1.1760x; 1.1760x over previous
"""Trainium2 Bass kernel for a GPT-2-style transformer block.

B=4, T=1024, C=768, H=12 heads (HD=64). 8 NeuronCores.

Sharding: 2 cores per batch sequence. Each core is fed a block-permuted
copy of its sequence (own query blocks at even block positions), computes
K/V for the full sequence locally (no collectives), runs causal attention
for its 512 query tokens with data-driven masks, and the full MLP for
those tokens. Host re-assembles the [B,T,C] output.

QKV/Wo/Wfc/Wproj matmuls run in bf16; residual/LN paths stay f32.
"""

import numpy as np
import ml_dtypes

P = 128
B, T, C, H = 4, 1024, 768, 12
HD = C // H        # 64
CJ = C // P        # 6 C-chunks
NT = T // P        # 8 token tiles
TQ = 512           # own query tokens per core
NQT = TQ // P      # 4 q slots
FC = 4 * C         # 3072
FCJ = FC // P      # 24
GELU_C = 0.035677408136300125  # sqrt(2/pi)*0.044715 -> tanh(c*x^4)
N_CORES = 8

_CACHED = {}


def _build_nc():
    import concourse.bass as bass
    from concourse import bacc, mybir
    import concourse.tile as tile
    from concourse.masks import make_identity
    from contextlib import ExitStack

    F32 = mybir.dt.float32
    F32R = mybir.dt.float32r
    BF16 = mybir.dt.bfloat16
    AF = mybir.ActivationFunctionType
    ALU = mybir.AluOpType

    nc = bacc.Bacc()

    xpe_d = nc.declare_dram_parameter("xpe", [TQ, C], F32, isOutput=False)
    xpo_d = nc.declare_dram_parameter("xpo", [TQ, C], BF16, isOutput=False)
    qg_d = nc.declare_dram_parameter("qg", [TQ], F32, isOutput=False)
    kg_d = nc.declare_dram_parameter("kg", [P, NT], F32, isOutput=False)
    wqkv_d = nc.declare_dram_parameter("wqkv", [C, 3 * C], BF16, isOutput=False)
    bqkv_d = nc.declare_dram_parameter("bqkv", [P, 18], F32, isOutput=False)
    bv_d = nc.declare_dram_parameter("bv", [C], F32, isOutput=False)
    wo_d = nc.declare_dram_parameter("wo", [C, C], BF16, isOutput=False)
    bo_d = nc.declare_dram_parameter("bo", [P, CJ], F32, isOutput=False)
    wfc_d = nc.declare_dram_parameter("wfc", [C, FC], BF16, isOutput=False)
    bfc_d = nc.declare_dram_parameter("bfc", [P, FCJ], F32, isOutput=False)
    wproj_d = nc.declare_dram_parameter("wproj", [FC, C], BF16, isOutput=False)
    bproj_d = nc.declare_dram_parameter("bproj", [P, CJ], F32, isOutput=False)
    out_d = nc.declare_dram_parameter("out", [TQ, C], F32, isOutput=True)

    def bcast_dma(engine, dst, dram_handle, offset, n):
        """DMA [n] DRAM vector broadcast across 128 partitions -> dst[128, n]."""
        ap = dram_handle[:]
        src = bass.AP(tensor=ap.tensor, offset=offset, ap=[[0, P], [1, n]])
        engine.dma_start(dst, src)

    with tile.TileContext(nc) as tc, ExitStack() as ctx:
        persist = ctx.enter_context(tc.tile_pool(name="persist", bufs=1))
        work = ctx.enter_context(tc.tile_pool(name="work", bufs=3))
        ln2c_p = ctx.enter_context(tc.tile_pool(name="ln2c_p", bufs=1))

        # ---------- constants ----------
        ident = persist.tile([P, P], F32, tag="ident")
        make_identity(nc, ident)
        ident_r = persist.tile([P, P], F32R, tag="identr")
        nc.vector.tensor_copy(ident_r, ident)
        eps_t = persist.tile([P, 1], F32, tag="eps")
        nc.vector.memset(eps_t, 1e-5)
        ones1_f32 = persist.tile([1, HD], F32, tag="ones1f")
        nc.vector.memset(ones1_f32, 1.0)
        ones1 = persist.tile([1, HD], BF16, tag="ones1")
        nc.vector.tensor_copy(ones1, ones1_f32)
        ones_col_f32 = persist.tile([P, 1], F32, tag="onescol")
        nc.vector.memset(ones_col_f32, 1.0)
        ones_col_r = persist.tile([P, 1], F32R, tag="onescolr")
        nc.vector.tensor_copy(ones_col_r, ones_col_f32)
        ones_row_bf = persist.tile([1, P], BF16, tag="onesrow")
        nc.vector.tensor_copy(ones_row_bf, ones_col_f32[0:1, 0:1].to_broadcast([1, P]))

        x_own = [persist.tile([P, C], F32, tag=f"xo{t}", name=f"xo{t}")
                 for t in range(NQT)]
        x1T = [persist.tile([P, TQ], F32R, tag=f"x1T{m}", name=f"x1T{m}")
               for m in range(CJ)]

        wq3 = wqkv_d[:, :].rearrange("(o p) n -> p o n", p=P)

        # small bias loads on gpsimd queue (host pre-transposed, contiguous)
        bqkv_po = persist.tile([P, 18], F32, tag="bqkv")
        nc.gpsimd.dma_start(bqkv_po, bqkv_d[:, :])
        bv_b = persist.tile([P, C], F32, tag="bv")
        bcast_dma(nc.gpsimd, bv_b, bv_d, 0, C)

        mask_cols = {0: (0, 128), 1: (0, 128), 2: (0, 256), 3: (0, 256),
                     4: (256, 128), 5: (256, 128), 6: (256, 256), 7: (256, 256)}
        masks = {}

        with tc.tile_pool(name="attn_live", bufs=1) as attn_live:
            # packed head-pair K/Q tiles: partitions [0:64]=head 2j, [64:128]=head 2j+1
            kTp = [attn_live.tile([P, T], BF16, tag=f"kTp{j}", name=f"kTp{j}")
                   for j in range(CJ)]
            qTp = [attn_live.tile([P, TQ], BF16, tag=f"qTp{j}", name=f"qTp{j}")
                   for j in range(CJ)]
            v_aug = [attn_live.tile([P, H, HD + 1], BF16, tag=f"vaug{t}",
                                    name=f"vaug{t}") for t in range(NT)]
            yT = [attn_live.tile([P, TQ], BF16, tag=f"yT{j}", name=f"yT{j}")
                  for j in range(CJ)]
            xlnT = [attn_live.tile([P, T], BF16, tag=f"xlnT{j}", name=f"xlnT{j}")
                    for j in range(CJ)]
            # weight tiles resident through the attention block
            vw = [attn_live.tile([P, CJ, 384], BF16, tag=f"vw{h}", name=f"vw{h}")
                  for h in range(2)]
            wtk = [attn_live.tile([P, CJ, 384], BF16, tag=f"wtk{k}",
                                  name=f"wtk{k}") for k in range(2)]
            wtq = [attn_live.tile([P, CJ, 384], BF16, tag=f"wtq{k}",
                                  name=f"wtq{k}") for k in range(2)]
            wo_t = attn_live.tile([P, CJ, C], BF16, tag="wo")
            wpj_t = attn_live.tile([P, FCJ, C], BF16, tag="wpj")

            with tc.tile_pool(name="ph12", bufs=1) as ph12, \
                 tc.tile_pool(name="ps_ln", bufs=2, space="PSUM") as ps_ln, \
                 tc.tile_pool(name="ps_v", bufs=3, space="PSUM") as ps_v:
                # x tiles: own (even permuted positions) f32; odd bf16
                x_sb = []
                for t in range(NT):
                    if t % 2 == 0:
                        x_sb.append(x_own[t // 2])
                    else:
                        xt = ph12.tile([P, C], BF16, tag=f"x{t}", name=f"x{t}")
                        x_sb.append(xt)
                nc.sync.dma_start(x_sb[0], xpe_d[0:P, :])
                nc.sync.dma_start(x_sb[1], xpo_d[0:P, :])
                for half in range(2):
                    nc.sync.dma_start(
                        vw[half],
                        wq3[:, :, 2 * C + half * 384: 2 * C + (half + 1) * 384])
                for t in range(2, NT):
                    src = xpe_d if t % 2 == 0 else xpo_d
                    nc.sync.dma_start(x_sb[t], src[(t // 2) * P:(t // 2 + 1) * P, :])
                for kp in range(2):
                    nc.sync.dma_start(
                        wtk[kp], wq3[:, :, C + kp * 384: C + (kp + 1) * 384])
                    nc.sync.dma_start(
                        wtq[kp], wq3[:, :, kp * 384:(kp + 1) * 384])
                nc.sync.dma_start(
                    wo_t, wo_d[:, :].rearrange("(o p) n -> p o n", p=P))
                nc.sync.dma_start(
                    wpj_t, wproj_d[:, :].rearrange("(o p) n -> p o n", p=P))

                # mask input loads (gpsimd queue, after bias loads)
                kg_po = ph12.tile([P, NT], F32, tag="kg")
                nc.gpsimd.dma_start(kg_po, kg_d[:, :])
                qg_b = ph12.tile([P, TQ], F32, tag="qgb")
                bcast_dma(nc.gpsimd, qg_b, qg_d, 0, TQ)

                # ---- LN1 + transpose + V projection, pipelined per tile ----
                for t in range(NT):
                    stats = work.tile([P, 3, 6], F32, tag="bnstats")
                    for g in range(3):
                        nc.vector.bn_stats(stats[:, g, :],
                                           x_sb[t][:, g * 256:(g + 1) * 256])
                    mv = work.tile([P, 2], F32, tag="bnmv")
                    nc.vector.bn_aggr(mv, stats)
                    rstd = work.tile([P, 1], F32, tag="rstd")
                    nc.scalar.activation(rstd, mv[:, 1:2], AF.Sqrt, bias=eps_t)
                    nc.vector.reciprocal(rstd, rstd)
                    nmr = work.tile([P, 1], F32, tag="nmr")
                    nc.vector.tensor_tensor(nmr, mv[:, 0:1], rstd, ALU.mult)
                    nc.vector.tensor_scalar(nmr, nmr, -1.0, None, ALU.mult)
                    xln = work.tile([P, C], F32R, tag="xln")
                    nc.scalar.activation(xln, x_sb[t], AF.Identity,
                                         bias=nmr, scale=rstd)
                    for j in range(CJ):
                        ptr = ps_ln.tile([P, P], F32R, tag="tr")
                        nc.tensor.transpose(ptr, xln[:, j * P:(j + 1) * P],
                                            ident_r)
                        dst = xlnT[j][:, t * P:(t + 1) * P]
                        if j % 2 == 0:
                            nc.vector.tensor_copy(dst, ptr)
                        else:
                            nc.scalar.copy(dst, ptr)
                    # V for this token tile: [128 tok, 384] x2, kc-outer to
                    # reuse the stationary xlnT chunk across halves
                    pmv = [ps_v.tile([P, 384], F32, tag="mmv", name=f"pmv{_h}")
                           for _h in range(2)]
                    for kc in range(CJ):
                        for half in range(2):
                            nc.tensor.matmul(
                                pmv[half],
                                xlnT[kc][:, t * P:(t + 1) * P],
                                vw[half][:, kc, :],
                                start=(kc == 0), stop=(kc == CJ - 1),
                                skip_group_check=True)
                    for half in range(2):
                        nc.vector.tensor_tensor(
                            v_aug[t][:, half * 6:(half + 1) * 6, 0:HD],
                            pmv[half].rearrange("p (h d) -> p h d", d=HD),
                            bv_b[:, half * 384:(half + 1) * 384].rearrange(
                                "p (h d) -> p h d", d=HD),
                            ALU.add)

                # masks + v_aug ones column (vector; before first AV use)
                for kc in range(NT):
                    off, w = mask_cols[kc]
                    m = persist.tile([P, w], BF16, tag=f"mask{kc}",
                                     name=f"mask{kc}")
                    nc.vector.tensor_scalar(
                        m, qg_b[:, off:off + w], kg_po[:, kc:kc + 1], None,
                        ALU.is_ge)
                    masks[kc] = m
                for t in range(NT):
                    nc.vector.tensor_copy(
                        v_aug[t][:, :, HD:HD + 1],
                        ones_col_f32.to_broadcast([P, H, 1]))

            # ---- K, Q (pipelined one tile ahead), attention per tile j ----
            with tc.tile_pool(name="att", bufs=3) as att, \
                 tc.tile_pool(name="ps_kq", bufs=3, space="PSUM") as ps_kq, \
                 tc.tile_pool(name="ps_sc", bufs=2, space="PSUM") as ps_sc, \
                 tc.tile_pool(name="ps_av", bufs=2, space="PSUM") as ps_av, \
                 tc.tile_pool(name="ps_bc", bufs=1, space="PSUM") as ps_bc:

                def kq_tile(j):
                    kp, jl = divmod(j, 3)
                    # K: both halves, kc-outer for stationary reuse
                    pmk = [ps_kq.tile([P, 512], F32, tag="kq",
                                      name=f"pmk{_h}") for _h in range(2)]
                    for kc in range(CJ):
                        for half in range(2):
                            nc.tensor.matmul(
                                pmk[half],
                                wtk[kp][:, kc, jl * P:(jl + 1) * P],
                                xlnT[kc][:, half * 512:(half + 1) * 512],
                                start=(kc == 0), stop=(kc == CJ - 1),
                                skip_group_check=True)
                    for half in range(2):
                        nc.scalar.activation(
                            kTp[j][:, half * 512:(half + 1) * 512],
                            pmk[half], AF.Identity,
                            bias=bqkv_po[:, CJ + j:CJ + j + 1])
                    # Q: own (even) blocks only
                    pmq = ps_kq.tile([P, 512], F32, tag="kq")
                    for kc in range(CJ):
                        own = xlnT[kc].rearrange(
                            "p (b c) -> p b c", c=P)[:, 0::2, :]
                        nc.tensor.matmul(
                            pmq, wtq[kp][:, kc, jl * P:(jl + 1) * P], own,
                            start=(kc == 0), stop=(kc == CJ - 1))
                    nc.scalar.activation(qTp[j], pmq, AF.Identity,
                                         bias=bqkv_po[:, j:j + 1])

                kq_tile(0)
                for j in range(CJ):
                    if j + 1 < CJ:
                        kq_tile(j + 1)
                    # ---- attention for the two heads of tile j ----
                    for hh in range(2):
                        h = 2 * j + hh
                        hs = slice(hh * HD, (hh + 1) * HD)
                        av = ps_av.tile([HD + 1, 512], F32, tag="av")
                        for kc in range(NT):
                            n0 = 0 if kc < 4 else 256
                            w = 512 - n0
                            sc = ps_sc.tile([P, 512], F32, tag="sc")
                            nc.tensor.matmul(
                                sc[:, 0:w],
                                kTp[j][hs, kc * P:(kc + 1) * P],
                                qTp[j][hs, n0:512],
                                start=True, stop=True)
                            ex = att.tile([P, 512], BF16, tag="exp")
                            nc.scalar.activation(ex[:, 0:w], sc[:, 0:w],
                                                 AF.Exp, scale=0.125)
                            off, wm = mask_cols[kc]
                            loc = off - n0
                            nc.vector.tensor_tensor(
                                ex[:, loc:loc + wm], ex[:, loc:loc + wm],
                                masks[kc], ALU.mult)
                            nc.tensor.matmul(
                                av[:, n0:512], v_aug[kc][:, h, :],
                                ex[:, 0:w],
                                start=(kc == 0), stop=(kc == NT - 1),
                                skip_group_check=True)
                        sums_bf = att.tile([1, 512], BF16, tag="sums")
                        nc.vector.tensor_copy(sums_bf, av[HD:HD + 1, :])
                        bc = ps_bc.tile([HD, 512], F32, tag="bc")
                        nc.tensor.matmul(bc, ones1, sums_bf,
                                         start=True, stop=True)
                        rb = att.tile([HD, 512], F32, tag="rb")
                        with nc.allow_low_precision(reason="softmax denom"):
                            nc.vector.reciprocal_approx_fast(rb, bc)
                        nc.vector.tensor_tensor(
                            yT[j][hs, :], av[0:HD, :], rb, ALU.mult)

            # ---- phase 4: x_own^T + Wo -> x1T, LN2 stats inline ----
            bo_po = persist.tile([P, CJ], F32, tag="bo")
            nc.gpsimd.dma_start(bo_po, bo_d[:, :])
            bfc_po = persist.tile([P, FCJ], F32, tag="bfc")
            nc.gpsimd.dma_start(bfc_po, bfc_d[:, :])
            bproj_po = persist.tile([P, CJ], F32, tag="bproj")
            nc.gpsimd.dma_start(bproj_po, bproj_d[:, :])

            with tc.tile_pool(name="ph4", bufs=3) as ph4, \
                 tc.tile_pool(name="ps_mm4", bufs=2, space="PSUM") as ps_mm4, \
                 tc.tile_pool(name="ps_st", bufs=1, space="PSUM") as ps_st, \
                 tc.tile_pool(name="ps_bc2", bufs=1, space="PSUM") as ps_bc2:
                mu_ps = ps_st.tile([1, TQ], F32, tag="mups", name="mups")
                sq_ps = ps_st.tile([1, TQ], F32, tag="sqps", name="sqps")
                for m in range(CJ):
                    pm = ps_mm4.tile([P, TQ], F32, tag="mm")
                    for t in range(NQT):
                        nc.tensor.matmul(
                            pm[:, t * P:(t + 1) * P],
                            x_own[t][:, m * P:(m + 1) * P], ident,
                            is_transpose=True,
                            start=(t == 0), stop=False,
                            skip_group_check=True)
                    for kc in range(CJ):
                        nc.tensor.matmul(
                            pm, wo_t[:, kc, m * P:(m + 1) * P], yT[kc],
                            start=False, stop=(kc == CJ - 1),
                            skip_group_check=True)
                    with nc.allow_low_precision(reason="residual f32r"):
                        nc.scalar.activation(x1T[m], pm, AF.Identity,
                                             bias=bo_po[:, m:m + 1])
                    nc.tensor.matmul(mu_ps, ones_col_r, x1T[m],
                                     start=(m == 0), stop=(m == CJ - 1))
                    sq = ph4.tile([P, TQ], F32R, tag="sq")
                    nc.scalar.activation(sq, x1T[m], AF.Square)
                    nc.tensor.matmul(sq_ps, ones_col_r, sq,
                                     start=(m == 0), stop=(m == CJ - 1))

                # LN2 scalars + broadcast + pre-normalized bf16 x1
                mu_f = ln2c_p.tile([1, TQ], F32, tag="muf")
                nc.vector.tensor_scalar(mu_f, mu_ps, 1.0 / C, None, ALU.mult)
                var_f = ln2c_p.tile([1, TQ], F32, tag="varf")
                nc.vector.tensor_scalar(var_f, sq_ps, 1.0 / C, None, ALU.mult)
                musq = ln2c_p.tile([1, TQ], F32, tag="musq")
                nc.vector.tensor_tensor(musq, mu_f, mu_f, ALU.mult)
                nc.vector.tensor_tensor(var_f, var_f, musq, ALU.subtract)
                rstd_f = ln2c_p.tile([1, TQ], F32, tag="rstdf")
                nc.scalar.activation(rstd_f, var_f, AF.Sqrt, bias=eps_t[0:1, :])
                nc.vector.reciprocal(rstd_f, rstd_f)
                murstd_f = ln2c_p.tile([1, TQ], F32, tag="murstdf")
                nc.vector.tensor_tensor(murstd_f, mu_f, rstd_f, ALU.mult)
                rstd_bf = ln2c_p.tile([1, TQ], BF16, tag="rstdbf")
                nc.vector.tensor_copy(rstd_bf, rstd_f)
                murstd_bf = ln2c_p.tile([1, TQ], BF16, tag="murstdbf")
                nc.vector.tensor_copy(murstd_bf, murstd_f)
                rstd_bc_ps = ps_bc2.tile([P, TQ], F32, tag="rstdbc",
                                         name="rstdbc")
                nc.tensor.matmul(rstd_bc_ps, ones_row_bf, rstd_bf,
                                 start=True, stop=True)
                murstd_bc_ps = ps_bc2.tile([P, TQ], F32, tag="murstdbc",
                                           name="murstdbc")
                nc.tensor.matmul(murstd_bc_ps, ones_row_bf, murstd_bf,
                                 start=True, stop=True)
                rstd_bc = ln2c_p.tile([P, TQ], F32, tag="rstdbcs")
                nc.vector.tensor_copy(rstd_bc, rstd_bc_ps)
                murstd_bc = ln2c_p.tile([P, TQ], F32, tag="murstdbcs")
                nc.vector.tensor_copy(murstd_bc, murstd_bc_ps)
                x1nb = [ln2c_p.tile([P, TQ], BF16, tag=f"x1nb{m}",
                                    name=f"x1nb{m}") for m in range(CJ)]
                for m in range(CJ):
                    tmp = ph4.tile([P, TQ], F32, tag="x1s")
                    nc.vector.tensor_tensor(tmp, x1T[m], rstd_bc, ALU.mult)
                    nc.vector.tensor_tensor(x1nb[m], tmp, murstd_bc,
                                            ALU.subtract)

        # ---------- phases 6-7: FC+gelu, proj+out ----------
        with tc.tile_pool(name="mlp_live", bufs=1) as mlp_live:
            h1T = [mlp_live.tile([P, TQ], BF16, tag=f"h1T{m}", name=f"h1T{m}")
                   for m in range(FCJ)]

            with tc.tile_pool(name="wfc_p", bufs=3) as wfc_p, \
                 tc.tile_pool(name="ph6", bufs=3) as ph6, \
                 tc.tile_pool(name="ps_mm6", bufs=3, space="PSUM") as ps_mm6:
                wfc3 = wfc_d[:, :].rearrange("(o p) n -> p o n", p=P)
                for m in range(FCJ):
                    if m % 4 == 0:
                        wt4 = wfc_p.tile([P, CJ, 512], BF16, tag="wfc")
                        nc.sync.dma_start(
                            wt4, wfc3[:, :, m * P:(m + 4) * P])
                    ml = m % 4
                    pm = ps_mm6.tile([P, TQ], F32, tag="mm")
                    for kc in range(CJ):
                        nc.tensor.matmul(pm, wt4[:, kc, ml * P:(ml + 1) * P],
                                         x1nb[kc],
                                         start=(kc == 0), stop=(kc == CJ - 1),
                                         skip_group_check=True)
                    xb = ph6.tile([P, TQ], F32, tag="xb")
                    nc.vector.tensor_scalar(xb, pm, bfc_po[:, m:m + 1], None,
                                            ALU.add)
                    sq = ph6.tile([P, TQ], F32, tag="gsq")
                    nc.scalar.activation(sq, xb, AF.Square)
                    q4 = ph6.tile([P, TQ], F32, tag="q4")
                    nc.vector.tensor_tensor(q4, sq, sq, ALU.mult)
                    u = ph6.tile([P, TQ], F32, tag="u")
                    nc.scalar.activation(u, q4, AF.Tanh, scale=GELU_C)
                    u5 = ph6.tile([P, TQ], F32, tag="u5")
                    nc.gpsimd.tensor_scalar(u5, u, 0.5, 0.5, ALU.mult, ALU.add)
                    nc.vector.tensor_tensor(h1T[m], xb, u5, ALU.mult)

            # ---------- phase 7: proj + residual -> out (m-outer) ----------
            with tc.tile_pool(name="ph7", bufs=2) as ph7, \
                 tc.tile_pool(name="out_p", bufs=1) as out_p, \
                 tc.tile_pool(name="ps_pj", bufs=3, space="PSUM") as ps_pj, \
                 tc.tile_pool(name="ps_tr7", bufs=2, space="PSUM") as ps_tr7:
                out_sb = [out_p.tile([P, C], F32, tag=f"osb{t}", name=f"osb{t}")
                          for t in range(NQT)]
                for m in range(CJ):
                    pm = ps_pj.tile([P, TQ], F32, tag="pj")
                    for kc in range(FCJ):
                        nc.tensor.matmul(
                            pm, wpj_t[:, kc, m * P:(m + 1) * P], h1T[kc],
                            start=(kc == 0), stop=(kc == FCJ - 1))
                    ojT = ph7.tile([P, TQ], F32R, tag="ojT")
                    nc.vector.tensor_scalar(
                        ojT, pm, bproj_po[:, m:m + 1], None, ALU.add)
                    nc.vector.tensor_tensor(ojT, ojT, x1T[m], ALU.add)
                    for t in range(NQT):
                        ptr = ps_tr7.tile([P, P], F32R, tag="tr")
                        nc.tensor.transpose(
                            ptr, ojT[:, t * P:(t + 1) * P], ident_r)
                        dst = out_sb[t][:, m * P:(m + 1) * P]
                        if m % 2 == 0:
                            nc.vector.tensor_copy(dst, ptr)
                        else:
                            nc.scalar.copy(dst, ptr)
                for t in range(NQT):
                    nc.sync.dma_start(out_d[t * P:(t + 1) * P, :], out_sb[t])

    nc.compile()
    return nc


def _get_nc():
    if "nc" not in _CACHED:
        _CACHED["nc"] = _build_nc()
    return _CACHED["nc"]


def _perm_blocks(p):
    return [p, 1 - p, 2 + p, 3 - p, 4 + p, 5 - p, 6 + p, 7 - p]


def _prepare(x, ln1_scale, ln1_bias, Wqkv, bqkv, Wo, bo,
             ln2_scale, ln2_bias, Wfc, bfc, Wproj, bproj):
    """Host-side prep: fold LN params into weights, permute qkv to
    [Q|K|V] layout, pre-transpose bias vectors, build per-core in_maps."""
    x = np.asarray(x, np.float32)
    Wqkv64 = np.asarray(Wqkv, np.float64)
    Wqkv64 = np.asarray(ln1_scale, np.float64)[:, None] * Wqkv64
    bqkv64 = np.asarray(bqkv, np.float64) + np.asarray(ln1_bias, np.float64) @ Wqkv64
    Wfc64 = np.asarray(Wfc, np.float64)
    Wfc64 = np.asarray(ln2_scale, np.float64)[:, None] * Wfc64
    bfc64 = np.asarray(bfc, np.float64) + np.asarray(ln2_bias, np.float64) @ Wfc64
    # Reference splits qkv per head: columns are [h0: q|k|v, h1: q|k|v, ...].
    colmap = np.arange(3 * C).reshape(H, 3, HD)
    qkv_perm = np.concatenate(
        [colmap[:, 0, :].ravel(), colmap[:, 1, :].ravel(), colmap[:, 2, :].ravel()])
    Wqkvp = Wqkv64.astype(np.float32)[:, qkv_perm]
    bqkvp = bqkv64.astype(np.float32)[qkv_perm]

    def po(v, cols):
        return np.ascontiguousarray(
            np.asarray(v, np.float32).reshape(cols, P).T)

    shared = {
        "wqkv": np.ascontiguousarray(Wqkvp.astype(ml_dtypes.bfloat16)),
        "bqkv": po(bqkvp, 18),
        "bv": np.ascontiguousarray(bqkvp[2 * C:]),
        "wo": np.ascontiguousarray(np.asarray(Wo, np.float32).astype(ml_dtypes.bfloat16)),
        "bo": po(bo, CJ),
        "wfc": np.ascontiguousarray(Wfc64.astype(ml_dtypes.bfloat16)),
        "bfc": po(bfc64.astype(np.float32), FCJ),
        "wproj": np.ascontiguousarray(np.asarray(Wproj, np.float32).astype(ml_dtypes.bfloat16)),
        "bproj": po(bproj, CJ),
    }
    in_maps = []
    own_toks = []
    for c in range(N_CORES):
        s, p = divmod(c, 2)
        blocks = _perm_blocks(p)
        tok = np.concatenate([np.arange(b * P, (b + 1) * P) for b in blocks])
        own = np.concatenate([np.arange(b * P, (b + 1) * P) for b in blocks[0::2]])
        odd = np.concatenate([np.arange(b * P, (b + 1) * P) for b in blocks[1::2]])
        own_toks.append((s, own))
        in_maps.append({
            "xpe": np.ascontiguousarray(x[s][own]),
            "xpo": np.ascontiguousarray(x[s][odd].astype(ml_dtypes.bfloat16)),
            "qg": own.astype(np.float32),
            "kg": po(tok.astype(np.float32), NT),
            **shared,
        })
    return in_maps, own_toks


def kernel(x, ln1_scale, ln1_bias, Wqkv, bqkv, Wo, bo,
           ln2_scale, ln2_bias, Wfc, bfc, Wproj, bproj):
    from concourse.bass_utils import run_bass_kernel_spmd

    in_maps, own_toks = _prepare(x, ln1_scale, ln1_bias, Wqkv, bqkv, Wo, bo,
                                 ln2_scale, ln2_bias, Wfc, bfc, Wproj, bproj)
    nc = _get_nc()
    res = run_bass_kernel_spmd(nc, in_maps, list(range(N_CORES)))

    out = np.empty((B, T, C), np.float32)
    for c in range(N_CORES):
        s, own = own_toks[c]
        out[s][own] = res.results[c]["out"]
    return out


# revision 13
# speedup vs baseline: 1.3645x; 1.1603x over previous
"""Trainium2 Bass kernel for a GPT-2-style transformer block.

B=4, T=1024, C=768, H=12 heads (HD=64). 8 NeuronCores.

Sharding: 2 cores per batch sequence. Each core is fed a block-permuted
copy of its sequence (own query blocks at even block positions), computes
K/V for the full sequence locally (no collectives), runs causal attention
for its 512 query tokens with data-driven masks, and the full MLP for
those tokens. Host re-assembles the [B,T,C] output.

QKV/Wo run bf16; FC/proj run fp8e4 DoubleRow (weights x32, acts x16,
rescaled on PSUM read); residual/LN paths stay f32.
"""

import numpy as np
import ml_dtypes

P = 128
B, T, C, H = 4, 1024, 768, 12
HD = C // H        # 64
CJ = C // P        # 6 C-chunks
NT = T // P        # 8 token tiles
TQ = 512           # own query tokens per core
NQT = TQ // P      # 4 q slots
FC = 4 * C         # 3072
FCJ = FC // P      # 24
GELU_C = 0.035677408136300125  # sqrt(2/pi)*0.044715 -> tanh(c*x^4)
WS = 32.0          # fp8 weight scale
XS = 16.0          # fp8 activation scale
N_CORES = 8

_CACHED = {}


def _build_nc():
    import concourse.bass as bass
    from concourse import bacc, mybir
    import concourse.tile as tile
    from concourse.masks import make_identity
    from contextlib import ExitStack

    F32 = mybir.dt.float32
    F32R = mybir.dt.float32r
    BF16 = mybir.dt.bfloat16
    FP8 = mybir.dt.float8e4
    AF = mybir.ActivationFunctionType
    ALU = mybir.AluOpType
    DR = mybir.MatmulPerfMode.DoubleRow

    nc = bacc.Bacc()

    xpe_d = nc.declare_dram_parameter("xpe", [TQ, C], F32, isOutput=False)
    xpo_d = nc.declare_dram_parameter("xpo", [TQ, C], BF16, isOutput=False)
    qg_d = nc.declare_dram_parameter("qg", [TQ], F32, isOutput=False)
    kg_d = nc.declare_dram_parameter("kg", [P, NT], F32, isOutput=False)
    wqkv_d = nc.declare_dram_parameter("wqkv", [C, 3 * C], BF16, isOutput=False)
    bqkv_d = nc.declare_dram_parameter("bqkv", [P, 18], F32, isOutput=False)
    wo_d = nc.declare_dram_parameter("wo", [C, C], BF16, isOutput=False)
    bo_d = nc.declare_dram_parameter("bo", [P, CJ], F32, isOutput=False)
    wfc_d = nc.declare_dram_parameter("wfc", [P, CJ, FC], FP8, isOutput=False)
    bfc_d = nc.declare_dram_parameter("bfc", [P, FCJ], F32, isOutput=False)
    wproj_d = nc.declare_dram_parameter("wproj", [P, FCJ, C], FP8, isOutput=False)
    bproj_d = nc.declare_dram_parameter("bproj", [P, CJ], F32, isOutput=False)
    out_d = nc.declare_dram_parameter("out", [TQ, C], F32, isOutput=True)

    def bcast_dma(engine, dst, dram_handle, offset, n):
        """DMA [n] DRAM vector broadcast across 128 partitions -> dst[128, n]."""
        ap = dram_handle[:]
        src = bass.AP(tensor=ap.tensor, offset=offset, ap=[[0, P], [1, n]])
        engine.dma_start(dst, src)

    with tile.TileContext(nc) as tc, ExitStack() as ctx:
        persist = ctx.enter_context(tc.tile_pool(name="persist", bufs=1))
        work = ctx.enter_context(tc.tile_pool(name="work", bufs=3))
        ln2c_p = ctx.enter_context(tc.tile_pool(name="ln2c_p", bufs=1))

        # ---------- constants ----------
        ident = persist.tile([P, P], F32, tag="ident")
        make_identity(nc, ident)
        ident_r = persist.tile([P, P], F32R, tag="identr")
        nc.vector.tensor_copy(ident_r, ident)
        eps_t = persist.tile([P, 1], F32, tag="eps")
        nc.vector.memset(eps_t, 1e-5)
        ones1_f32 = persist.tile([1, HD], F32, tag="ones1f")
        nc.vector.memset(ones1_f32, 1.0)
        ones1 = persist.tile([1, HD], BF16, tag="ones1")
        nc.vector.tensor_copy(ones1, ones1_f32)
        ones_col_f32 = persist.tile([P, 1], F32, tag="onescol")
        nc.vector.memset(ones_col_f32, 1.0)
        ones_col_r = persist.tile([P, 1], F32R, tag="onescolr")
        nc.vector.tensor_copy(ones_col_r, ones_col_f32)
        ones_row_bf = persist.tile([1, P], BF16, tag="onesrow")
        nc.vector.tensor_copy(ones_row_bf, ones_col_f32[0:1, 0:1].to_broadcast([1, P]))

        # per-head q tiles (zero-padded other half; K=128 score matmuls)
        qT = [persist.tile([P, TQ], BF16, tag=f"qT{h}", name=f"qT{h}")
              for h in range(H)]
        for h in range(H):
            zlo = slice(0, HD) if h % 2 else slice(HD, P)
            nc.vector.memset(qT[h][zlo, :], 0.0)

        x_own = [persist.tile([P, C], F32, tag=f"xo{t}", name=f"xo{t}")
                 for t in range(NQT)]
        x1T = [persist.tile([P, TQ], F32R, tag=f"x1T{m}", name=f"x1T{m}")
               for m in range(CJ)]

        wq3 = wqkv_d[:, :].rearrange("(o p) n -> p o n", p=P)

        # small bias loads on gpsimd queue (host pre-transposed, contiguous)
        bqkv_po = persist.tile([P, 18], F32, tag="bqkv")
        nc.gpsimd.dma_start(bqkv_po, bqkv_d[:, :])

        mask_cols = {0: (0, 128), 1: (0, 128), 2: (0, 256), 3: (0, 256),
                     4: (256, 128), 5: (256, 128), 6: (256, 256), 7: (256, 256)}
        masks = {}

        with tc.tile_pool(name="attn_live", bufs=1) as attn_live:
            # packed head-pair K tiles: partitions [0:64]=head 2j, [64:128]=head 2j+1
            kTp = [attn_live.tile([P, T], BF16, tag=f"kTp{j}", name=f"kTp{j}")
                   for j in range(CJ)]
            v_aug = [attn_live.tile([P, H, HD + 1], BF16, tag=f"vaug{t}",
                                    name=f"vaug{t}") for t in range(NT)]
            yT = [attn_live.tile([P, TQ], BF16, tag=f"yT{j}", name=f"yT{j}")
                  for j in range(CJ)]
            xlnT = [attn_live.tile([P, T], BF16, tag=f"xlnT{j}", name=f"xlnT{j}")
                    for j in range(CJ)]
            # weight tiles resident through the attention block
            vw = [attn_live.tile([P, CJ, 384], BF16, tag=f"vw{h}", name=f"vw{h}")
                  for h in range(2)]
            wtk = [attn_live.tile([P, CJ, 384], BF16, tag=f"wtk{k}",
                                  name=f"wtk{k}") for k in range(2)]
            wtq = [attn_live.tile([P, CJ, 384], BF16, tag=f"wtq{k}",
                                  name=f"wtq{k}") for k in range(2)]
            wo_t = attn_live.tile([P, CJ, C], BF16, tag="wo")
            wpj_t = attn_live.tile([P, FCJ // 2, 2, C], FP8, tag="wpj")

            with tc.tile_pool(name="ph12", bufs=1) as ph12, \
                 tc.tile_pool(name="ps_ln", bufs=2, space="PSUM") as ps_ln, \
                 tc.tile_pool(name="ps_v", bufs=3, space="PSUM") as ps_v:
                # x tiles: own (even permuted positions) f32; odd bf16
                x_sb = []
                for t in range(NT):
                    if t % 2 == 0:
                        x_sb.append(x_own[t // 2])
                    else:
                        xt = ph12.tile([P, C], BF16, tag=f"x{t}", name=f"x{t}")
                        x_sb.append(xt)
                nc.sync.dma_start(x_sb[0], xpe_d[0:P, :])
                nc.sync.dma_start(x_sb[1], xpo_d[0:P, :])
                # mask inputs early on the fast sync queue
                kg_po = ph12.tile([P, NT], F32, tag="kg")
                nc.sync.dma_start(kg_po, kg_d[:, :])
                qg_b = ph12.tile([P, TQ], F32, tag="qgb")
                bcast_dma(nc.sync, qg_b, qg_d, 0, TQ)
                for half in range(2):
                    nc.sync.dma_start(
                        vw[half],
                        wq3[:, :, 2 * C + half * 384: 2 * C + (half + 1) * 384])
                for t in range(2, NT):
                    src = xpe_d if t % 2 == 0 else xpo_d
                    nc.sync.dma_start(x_sb[t], src[(t // 2) * P:(t // 2 + 1) * P, :])
                for kp in range(2):
                    nc.sync.dma_start(
                        wtk[kp], wq3[:, :, C + kp * 384: C + (kp + 1) * 384])
                    nc.sync.dma_start(
                        wtq[kp], wq3[:, :, kp * 384:(kp + 1) * 384])
                nc.sync.dma_start(
                    wo_t, wo_d[:, :].rearrange("(o p) n -> p o n", p=P))
                nc.sync.dma_start(wpj_t, wproj_d[:, :, :])

                # ---- LN1 + transpose + V projection, pipelined per tile ----
                for t in range(NT):
                    stats = work.tile([P, 3, 6], F32, tag="bnstats")
                    for g in range(3):
                        nc.vector.bn_stats(stats[:, g, :],
                                           x_sb[t][:, g * 256:(g + 1) * 256])
                    mv = work.tile([P, 2], F32, tag="bnmv")
                    nc.vector.bn_aggr(mv, stats)
                    rstd = work.tile([P, 1], F32, tag="rstd")
                    nc.scalar.activation(rstd, mv[:, 1:2], AF.Sqrt, bias=eps_t)
                    nc.vector.reciprocal(rstd, rstd)
                    nmr = work.tile([P, 1], F32, tag="nmr")
                    nc.vector.scalar_tensor_tensor(
                        nmr, mv[:, 0:1], -1.0, rstd, ALU.mult, ALU.mult)
                    xln = work.tile([P, C], F32R, tag="xln")
                    nc.scalar.activation(xln, x_sb[t], AF.Identity,
                                         bias=nmr, scale=rstd)
                    for j in range(CJ):
                        ptr = ps_ln.tile([P, P], F32R, tag="tr")
                        nc.tensor.transpose(ptr, xln[:, j * P:(j + 1) * P],
                                            ident_r)
                        dst = xlnT[j][:, t * P:(t + 1) * P]
                        if j in (0, 3):
                            nc.vector.tensor_copy(dst, ptr)
                        else:
                            nc.scalar.copy(dst, ptr)
                    # V for this token tile (V bias folded into bo host-side)
                    pmv = [ps_v.tile([P, 384], F32, tag="mmv", name=f"pmv{_h}")
                           for _h in range(2)]
                    for kc in range(CJ):
                        for half in range(2):
                            nc.tensor.matmul(
                                pmv[half],
                                xlnT[kc][:, t * P:(t + 1) * P],
                                vw[half][:, kc, :],
                                start=(kc == 0), stop=(kc == CJ - 1),
                                skip_group_check=True)
                    nc.vector.tensor_copy(
                        v_aug[t][:, 0:6, 0:HD],
                        pmv[0].rearrange("p (h d) -> p h d", d=HD))
                    nc.scalar.copy(
                        v_aug[t][:, 6:12, 0:HD],
                        pmv[1].rearrange("p (h d) -> p h d", d=HD))

                # masks + v_aug ones column (before first AV use)
                for kc in range(NT):
                    off, w = mask_cols[kc]
                    m = persist.tile([P, w], BF16, tag=f"mask{kc}",
                                     name=f"mask{kc}")
                    nc.vector.tensor_scalar(
                        m, qg_b[:, off:off + w], kg_po[:, kc:kc + 1], None,
                        ALU.is_ge)
                    masks[kc] = m
                for t in range(NT):
                    nc.vector.tensor_copy(
                        v_aug[t][:, :, HD:HD + 1],
                        ones_col_f32.to_broadcast([P, H, 1]))

            # ---- K, Q (pipelined one tile ahead), attention per tile j ----
            with tc.tile_pool(name="att", bufs=3) as att, \
                 tc.tile_pool(name="ps_kq", bufs=3, space="PSUM") as ps_kq, \
                 tc.tile_pool(name="ps_sc", bufs=2, space="PSUM") as ps_sc, \
                 tc.tile_pool(name="ps_av", bufs=2, space="PSUM") as ps_av, \
                 tc.tile_pool(name="ps_bc", bufs=1, space="PSUM") as ps_bc:

                def kq_tile(j):
                    kp, jl = divmod(j, 3)
                    # K: both halves, kc-outer for stationary reuse
                    pmk = [ps_kq.tile([P, 512], F32, tag="kq",
                                      name=f"pmk{_h}") for _h in range(2)]
                    for kc in range(CJ):
                        for half in range(2):
                            nc.tensor.matmul(
                                pmk[half],
                                wtk[kp][:, kc, jl * P:(jl + 1) * P],
                                xlnT[kc][:, half * 512:(half + 1) * 512],
                                start=(kc == 0), stop=(kc == CJ - 1),
                                skip_group_check=True)
                    for half in range(2):
                        nc.scalar.activation(
                            kTp[j][:, half * 512:(half + 1) * 512],
                            pmk[half], AF.Identity,
                            bias=bqkv_po[:, CJ + j:CJ + j + 1])
                    # Q: own (even) blocks only; per-head zero-padded tiles
                    pmq = ps_kq.tile([P, 512], F32, tag="kq")
                    for kc in range(CJ):
                        own = xlnT[kc].rearrange(
                            "p (b c) -> p b c", c=P)[:, 0::2, :]
                        nc.tensor.matmul(
                            pmq, wtq[kp][:, kc, jl * P:(jl + 1) * P], own,
                            start=(kc == 0), stop=(kc == CJ - 1))
                    for hh in range(2):
                        hs = slice(hh * HD, (hh + 1) * HD)
                        nc.scalar.activation(
                            qT[2 * j + hh][hs, :], pmq[hs, :], AF.Identity,
                            bias=bqkv_po[hs, j:j + 1])

                kq_tile(0)
                for j in range(CJ):
                    if j + 1 < CJ:
                        kq_tile(j + 1)
                    # ---- attention for the two heads of tile j ----
                    for hh in range(2):
                        h = 2 * j + hh
                        av = ps_av.tile([HD + 1, 512], F32, tag="av")
                        prev = None
                        for kc in range(NT):
                            n0 = 0 if kc < 4 else 256
                            w = 512 - n0
                            sc = ps_sc.tile([P, 512], F32, tag="sc")
                            nc.tensor.matmul(
                                sc[:, 0:w],
                                kTp[j][:, kc * P:(kc + 1) * P],
                                qT[h][:, n0:512],
                                start=True, stop=True)
                            ex = att.tile([P, 512], BF16, tag="exp")
                            nc.scalar.activation(ex[:, 0:w], sc[:, 0:w],
                                                 AF.Exp, scale=0.125)
                            off, wm = mask_cols[kc]
                            loc = off - n0
                            nc.gpsimd.tensor_tensor(
                                ex[:, loc:loc + wm], ex[:, loc:loc + wm],
                                masks[kc], ALU.mult)
                            if prev is not None:
                                pkc, pn0, pw, pex = prev
                                nc.tensor.matmul(
                                    av[:, pn0:512], v_aug[pkc][:, h, :],
                                    pex[:, 0:pw],
                                    start=(pkc == 0), stop=False,
                                    skip_group_check=True)
                            prev = (kc, n0, w, ex)
                        pkc, pn0, pw, pex = prev
                        nc.tensor.matmul(
                            av[:, pn0:512], v_aug[pkc][:, h, :],
                            pex[:, 0:pw],
                            start=False, stop=True,
                            skip_group_check=True)
                        sums_bf = att.tile([1, 512], BF16, tag="sums")
                        nc.vector.tensor_copy(sums_bf, av[HD:HD + 1, :])
                        bc = ps_bc.tile([HD, 512], F32, tag="bc")
                        nc.tensor.matmul(bc, ones1, sums_bf,
                                         start=True, stop=True)
                        rb = att.tile([HD, 512], F32, tag="rb")
                        with nc.allow_low_precision(reason="softmax denom"):
                            nc.vector.reciprocal_approx_fast(rb, bc)
                        hs = slice(hh * HD, (hh + 1) * HD)
                        nc.vector.tensor_tensor(
                            yT[j][hs, :], av[0:HD, :], rb, ALU.mult)

            # ---- phase 4: x_own^T + Wo -> x1T, LN2 stats inline ----
            bo_po = persist.tile([P, CJ], F32, tag="bo")
            nc.gpsimd.dma_start(bo_po, bo_d[:, :])
            bfc_po = persist.tile([P, FCJ], F32, tag="bfc")
            nc.gpsimd.dma_start(bfc_po, bfc_d[:, :])
            bproj_po = persist.tile([P, CJ], F32, tag="bproj")
            nc.gpsimd.dma_start(bproj_po, bproj_d[:, :])

            with tc.tile_pool(name="ph4", bufs=3) as ph4, \
                 tc.tile_pool(name="ps_mm4", bufs=2, space="PSUM") as ps_mm4, \
                 tc.tile_pool(name="ps_st", bufs=1, space="PSUM") as ps_st, \
                 tc.tile_pool(name="ps_bc2", bufs=1, space="PSUM") as ps_bc2:
                mu_ps = ps_st.tile([1, TQ], F32, tag="mups", name="mups")
                sq_ps = ps_st.tile([1, TQ], F32, tag="sqps", name="sqps")
                for m in range(CJ):
                    pm = ps_mm4.tile([P, TQ], F32, tag="mm")
                    for t in range(NQT):
                        nc.tensor.matmul(
                            pm[:, t * P:(t + 1) * P],
                            x_own[t][:, m * P:(m + 1) * P], ident,
                            is_transpose=True,
                            start=(t == 0), stop=False,
                            skip_group_check=True)
                    for kc in range(CJ):
                        nc.tensor.matmul(
                            pm, wo_t[:, kc, m * P:(m + 1) * P], yT[kc],
                            start=False, stop=(kc == CJ - 1),
                            skip_group_check=True)
                    with nc.allow_low_precision(reason="residual f32r"):
                        nc.scalar.activation(x1T[m], pm, AF.Identity,
                                             bias=bo_po[:, m:m + 1])
                    nc.tensor.matmul(mu_ps, ones_col_r, x1T[m],
                                     start=(m == 0), stop=(m == CJ - 1))
                    sq = ph4.tile([P, TQ], F32R, tag="sq")
                    nc.scalar.activation(sq, x1T[m], AF.Square)
                    nc.tensor.matmul(sq_ps, ones_col_r, sq,
                                     start=(m == 0), stop=(m == CJ - 1))

                # LN2 scalars: var = sq/C - (mu/C)^2; rstd = 1/sqrt(var+eps)
                mu_s = ln2c_p.tile([1, TQ], F32, tag="mus")
                nc.vector.tensor_scalar(mu_s, mu_ps, 1.0 / C, None, ALU.mult)
                musq = ln2c_p.tile([1, TQ], F32, tag="musq")
                nc.vector.tensor_tensor(musq, mu_s, mu_s, ALU.mult)
                var_f = ln2c_p.tile([1, TQ], F32, tag="varf")
                nc.vector.scalar_tensor_tensor(
                    var_f, sq_ps, 1.0 / C, musq, ALU.mult, ALU.subtract)
                rstd_f = ln2c_p.tile([1, TQ], F32, tag="rstdf")
                nc.scalar.activation(rstd_f, var_f, AF.Sqrt, bias=eps_t[0:1, :])
                nc.vector.reciprocal(rstd_f, rstd_f)
                # scaled (xXS) bf16 rows for broadcast
                rstd_bf = ln2c_p.tile([1, TQ], BF16, tag="rstdbf")
                nc.vector.tensor_scalar(rstd_bf, rstd_f, XS, None, ALU.mult)
                murstd_bf = ln2c_p.tile([1, TQ], BF16, tag="murstdbf")
                nc.vector.scalar_tensor_tensor(
                    murstd_bf, mu_s, XS, rstd_f, ALU.mult, ALU.mult)
                rstd_bc = ps_bc2.tile([P, TQ], F32, tag="rstdbc", name="rstdbc")
                nc.tensor.matmul(rstd_bc, ones_row_bf, rstd_bf,
                                 start=True, stop=True)
                murstd_bc = ps_bc2.tile([P, TQ], F32, tag="murstdbc",
                                        name="murstdbc")
                nc.tensor.matmul(murstd_bc, ones_row_bf, murstd_bf,
                                 start=True, stop=True)
                # pre-normalized fp8 x1 (x XS), paired for DoubleRow
                x1p = [ln2c_p.tile([P, 2, TQ], FP8, tag=f"x1p{c}",
                                   name=f"x1p{c}") for c in range(CJ // 2)]
                for m in range(CJ):
                    tmp = ph4.tile([P, TQ], F32, tag="x1s")
                    nc.vector.tensor_tensor(tmp, x1T[m], rstd_bc, ALU.mult)
                    nc.vector.tensor_tensor(x1p[m // 2][:, m % 2, :], tmp,
                                            murstd_bc, ALU.subtract)

        # ---------- phases 6-7: FC+gelu, proj+out ----------
        with tc.tile_pool(name="mlp_live", bufs=1) as mlp_live:
            h1p = [mlp_live.tile([P, 2, TQ], FP8, tag=f"h1p{m}",
                                 name=f"h1p{m}") for m in range(FCJ // 2)]

            with tc.tile_pool(name="wfc_p", bufs=3) as wfc_p, \
                 tc.tile_pool(name="ph6", bufs=3) as ph6, \
                 tc.tile_pool(name="ps_mm6", bufs=3, space="PSUM") as ps_mm6:
                for m in range(FCJ):
                    if m % 4 == 0:
                        wt4 = wfc_p.tile([P, CJ // 2, 2, 512], FP8, tag="wfc")
                        nc.sync.dma_start(
                            wt4, wfc_d[:, :, m * P:(m + 4) * P])
                    ml = m % 4
                    pm = ps_mm6.tile([P, TQ], F32, tag="mm")
                    for c2 in range(CJ // 2):
                        nc.tensor.matmul(
                            pm, wt4[:, c2, :, ml * P:(ml + 1) * P], x1p[c2],
                            start=(c2 == 0), stop=(c2 == CJ // 2 - 1),
                            perf_mode=DR, skip_group_check=True)
                    xb = ph6.tile([P, TQ], F32, tag="xb")
                    nc.vector.tensor_scalar(xb, pm, 1.0 / (WS * XS),
                                            bfc_po[:, m:m + 1],
                                            ALU.mult, ALU.add)
                    sq = ph6.tile([P, TQ], F32, tag="gsq")
                    nc.scalar.activation(sq, xb, AF.Square)
                    q4 = ph6.tile([P, TQ], F32, tag="q4")
                    nc.vector.tensor_tensor(q4, sq, sq, ALU.mult)
                    u = ph6.tile([P, TQ], F32, tag="u")
                    nc.scalar.activation(u, q4, AF.Sigmoid, scale=2 * GELU_C)
                    nc.vector.scalar_tensor_tensor(
                        h1p[m // 2][:, m % 2, :], xb, XS, u,
                        ALU.mult, ALU.mult)

            # ---------- phase 7: proj + residual -> out (m-outer) ----------
            with tc.tile_pool(name="ph7", bufs=2) as ph7, \
                 tc.tile_pool(name="out_p", bufs=1) as out_p, \
                 tc.tile_pool(name="ps_pj", bufs=3, space="PSUM") as ps_pj, \
                 tc.tile_pool(name="ps_tr7", bufs=2, space="PSUM") as ps_tr7:
                out_sb = [out_p.tile([P, C], F32, tag=f"osb{t}", name=f"osb{t}")
                          for t in range(NQT)]
                for m in range(CJ):
                    pm = ps_pj.tile([P, TQ], F32, tag="pj")
                    for kc2 in range(FCJ // 2):
                        nc.tensor.matmul(
                            pm, wpj_t[:, kc2, :, m * P:(m + 1) * P], h1p[kc2],
                            start=(kc2 == 0), stop=(kc2 == FCJ // 2 - 1),
                            perf_mode=DR)
                    ojT = ph7.tile([P, TQ], F32R, tag="ojT")
                    nc.vector.tensor_scalar(ojT, pm, 1.0 / (WS * XS),
                                            bproj_po[:, m:m + 1],
                                            ALU.mult, ALU.add)
                    nc.vector.tensor_tensor(ojT, ojT, x1T[m], ALU.add)
                    for t in range(NQT):
                        ptr = ps_tr7.tile([P, P], F32R, tag="tr")
                        nc.tensor.transpose(
                            ptr, ojT[:, t * P:(t + 1) * P], ident_r)
                        dst = out_sb[t][:, m * P:(m + 1) * P]
                        if m % 2 == 0:
                            nc.vector.tensor_copy(dst, ptr)
                        else:
                            nc.scalar.copy(dst, ptr)
                for t in range(NQT):
                    nc.sync.dma_start(out_d[t * P:(t + 1) * P, :], out_sb[t])

    nc.compile()
    return nc


def _get_nc():
    if "nc" not in _CACHED:
        _CACHED["nc"] = _build_nc()
    return _CACHED["nc"]


def _perm_blocks(p):
    return [p, 1 - p, 2 + p, 3 - p, 4 + p, 5 - p, 6 + p, 7 - p]


def _prepare(x, ln1_scale, ln1_bias, Wqkv, bqkv, Wo, bo,
             ln2_scale, ln2_bias, Wfc, bfc, Wproj, bproj):
    """Host-side prep: fold LN params into weights, permute qkv to
    [Q|K|V] layout, pre-transpose bias vectors, build per-core in_maps."""
    FP8NP = ml_dtypes.float8_e4m3
    x = np.asarray(x, np.float32)
    Wqkv64 = np.asarray(Wqkv, np.float64)
    Wqkv64 = np.asarray(ln1_scale, np.float64)[:, None] * Wqkv64
    bqkv64 = np.asarray(bqkv, np.float64) + np.asarray(ln1_bias, np.float64) @ Wqkv64
    Wfc64 = np.asarray(Wfc, np.float64)
    Wfc64 = np.asarray(ln2_scale, np.float64)[:, None] * Wfc64
    bfc64 = np.asarray(bfc, np.float64) + np.asarray(ln2_bias, np.float64) @ Wfc64
    # Reference splits qkv per head: columns are [h0: q|k|v, h1: q|k|v, ...].
    colmap = np.arange(3 * C).reshape(H, 3, HD)
    qkv_perm = np.concatenate(
        [colmap[:, 0, :].ravel(), colmap[:, 1, :].ravel(), colmap[:, 2, :].ravel()])
    Wqkvp = Wqkv64.astype(np.float32)[:, qkv_perm]
    bqkvp = bqkv64.astype(np.float32)[qkv_perm]
    # fold V bias through Wo into bo (softmax weights sum to 1)
    Wo64 = np.asarray(Wo, np.float64)
    bo64 = np.asarray(bo, np.float64) + bqkv64[qkv_perm][2 * C:] @ Wo64

    def po(v, cols):
        return np.ascontiguousarray(
            np.asarray(v, np.float32).reshape(cols, P).T)

    def w8(w, rows):
        # [rows*P, n] -> [P, rows, n] scaled fp8
        w = np.asarray(w, np.float64) * WS
        return np.ascontiguousarray(
            w.reshape(rows, P, -1).transpose(1, 0, 2).astype(FP8NP))

    shared = {
        "wqkv": np.ascontiguousarray(Wqkvp.astype(ml_dtypes.bfloat16)),
        "bqkv": po(bqkvp, 18),
        "wo": np.ascontiguousarray(np.asarray(Wo, np.float32).astype(ml_dtypes.bfloat16)),
        "bo": po(bo64.astype(np.float32), CJ),
        "wfc": w8(Wfc64, CJ),
        "bfc": po(bfc64.astype(np.float32), FCJ),
        "wproj": w8(Wproj, FCJ),
        "bproj": po(bproj, CJ),
    }
    in_maps = []
    own_toks = []
    for c in range(N_CORES):
        s, p = divmod(c, 2)
        blocks = _perm_blocks(p)
        tok = np.concatenate([np.arange(b * P, (b + 1) * P) for b in blocks])
        own = np.concatenate([np.arange(b * P, (b + 1) * P) for b in blocks[0::2]])
        odd = np.concatenate([np.arange(b * P, (b + 1) * P) for b in blocks[1::2]])
        own_toks.append((s, own))
        in_maps.append({
            "xpe": np.ascontiguousarray(x[s][own]),
            "xpo": np.ascontiguousarray(x[s][odd].astype(ml_dtypes.bfloat16)),
            "qg": own.astype(np.float32),
            "kg": po(tok.astype(np.float32), NT),
            **shared,
        })
    return in_maps, own_toks


def kernel(x, ln1_scale, ln1_bias, Wqkv, bqkv, Wo, bo,
           ln2_scale, ln2_bias, Wfc, bfc, Wproj, bproj):
    from concourse.bass_utils import run_bass_kernel_spmd

    in_maps, own_toks = _prepare(x, ln1_scale, ln1_bias, Wqkv, bqkv, Wo, bo,
                                 ln2_scale, ln2_bias, Wfc, bfc, Wproj, bproj)
    nc = _get_nc()
    res = run_bass_kernel_spmd(nc, in_maps, list(range(N_CORES)))

    out = np.empty((B, T, C), np.float32)
    for c in range(N_CORES):
        s, own = own_toks[c]
        out[s][own] = res.results[c]["out"]
    return out


# revision 18
# speedup vs baseline: 1.3831x; 1.0136x over previous
"""Trainium2 Bass kernel for a GPT-2-style transformer block.

B=4, T=1024, C=768, H=12 heads (HD=64). 8 NeuronCores.

Sharding: 2 cores per batch sequence. Each core is fed a block-permuted
copy of its sequence (own query blocks at even block positions), computes
K/V for the full sequence locally (no collectives), runs causal attention
for its 512 query tokens with data-driven masks, and the full MLP for
those tokens. Host re-assembles the [B,T,C] output.

QKV/Wo run bf16; FC/proj run fp8e4 DoubleRow (weights x32, acts x16,
rescaled on PSUM read); residual/LN paths stay f32.
"""

import numpy as np
import ml_dtypes

P = 128
B, T, C, H = 4, 1024, 768, 12
HD = C // H        # 64
CJ = C // P        # 6 C-chunks
NT = T // P        # 8 token tiles
TQ = 512           # own query tokens per core
NQT = TQ // P      # 4 q slots
FC = 4 * C         # 3072
FCJ = FC // P      # 24
GELU_C = 0.035677408136300125  # sqrt(2/pi)*0.044715 -> tanh(c*x^4)
WS = 32.0          # fp8 weight scale
XS = 16.0          # fp8 activation scale
N_CORES = 8

_CACHED = {}


def _build_nc():
    import concourse.bass as bass
    from concourse import bacc, mybir
    import concourse.tile as tile
    from concourse.masks import make_identity
    from contextlib import ExitStack

    F32 = mybir.dt.float32
    F32R = mybir.dt.float32r
    BF16 = mybir.dt.bfloat16
    FP8 = mybir.dt.float8e4
    AF = mybir.ActivationFunctionType
    ALU = mybir.AluOpType
    DR = mybir.MatmulPerfMode.DoubleRow

    nc = bacc.Bacc()

    xpe_d = nc.declare_dram_parameter("xpe", [TQ, C], F32, isOutput=False)
    xpo_d = nc.declare_dram_parameter("xpo", [TQ, C], BF16, isOutput=False)
    qg_d = nc.declare_dram_parameter("qg", [TQ], F32, isOutput=False)
    kg_d = nc.declare_dram_parameter("kg", [P, NT], F32, isOutput=False)
    wqkv_d = nc.declare_dram_parameter("wqkv", [P, CJ, 3 * C], FP8, isOutput=False)
    bqkv_d = nc.declare_dram_parameter("bqkv", [P, 18], F32, isOutput=False)
    wo_d = nc.declare_dram_parameter("wo", [C, C], BF16, isOutput=False)
    bo_d = nc.declare_dram_parameter("bo", [P, CJ], F32, isOutput=False)
    wfc_d = nc.declare_dram_parameter("wfc", [P, CJ, FC], FP8, isOutput=False)
    bfc_d = nc.declare_dram_parameter("bfc", [P, FCJ], F32, isOutput=False)
    wproj_d = nc.declare_dram_parameter("wproj", [P, FCJ, C], FP8, isOutput=False)
    bproj_d = nc.declare_dram_parameter("bproj", [P, CJ], F32, isOutput=False)
    out_d = nc.declare_dram_parameter("out", [TQ, C], F32, isOutput=True)

    def bcast_dma(engine, dst, dram_handle, offset, n):
        """DMA [n] DRAM vector broadcast across 128 partitions -> dst[128, n]."""
        ap = dram_handle[:]
        src = bass.AP(tensor=ap.tensor, offset=offset, ap=[[0, P], [1, n]])
        engine.dma_start(dst, src)

    with tile.TileContext(nc) as tc, ExitStack() as ctx:
        persist = ctx.enter_context(tc.tile_pool(name="persist", bufs=1))
        work = ctx.enter_context(tc.tile_pool(name="work", bufs=3))
        ln2c_p = ctx.enter_context(tc.tile_pool(name="ln2c_p", bufs=1))

        # ---------- constants ----------
        ident = persist.tile([P, P], F32, tag="ident")
        make_identity(nc, ident)
        ident_r = persist.tile([P, P], F32R, tag="identr")
        nc.vector.tensor_copy(ident_r, ident)
        eps_t = persist.tile([P, 1], F32, tag="eps")
        nc.vector.memset(eps_t, 1e-5)
        ones1_f32 = persist.tile([1, HD], F32, tag="ones1f")
        nc.vector.memset(ones1_f32, 1.0)
        ones1 = persist.tile([1, HD], BF16, tag="ones1")
        nc.vector.tensor_copy(ones1, ones1_f32)
        ones_col_f32 = persist.tile([P, 1], F32, tag="onescol")
        nc.vector.memset(ones_col_f32, 1.0)
        ones_col_r = persist.tile([P, 1], F32R, tag="onescolr")
        nc.vector.tensor_copy(ones_col_r, ones_col_f32)
        ones_row_bf = persist.tile([1, P], BF16, tag="onesrow")
        nc.vector.tensor_copy(ones_row_bf, ones_col_f32[0:1, 0:1].to_broadcast([1, P]))

        # per-head q tiles (zero-padded other half; K=128 score matmuls)
        qT = [persist.tile([P, TQ], BF16, tag=f"qT{h}", name=f"qT{h}")
              for h in range(H)]
        for h in range(H):
            zlo = slice(0, HD) if h % 2 else slice(HD, P)
            nc.vector.memset(qT[h][zlo, :], 0.0)

        x_own = [persist.tile([P, C], F32, tag=f"xo{t}", name=f"xo{t}")
                 for t in range(NQT)]
        x1T = [persist.tile([P, TQ], F32R, tag=f"x1T{m}", name=f"x1T{m}")
               for m in range(CJ)]

        wpj_t = persist.tile([P, FCJ // 2, 2, C], FP8, tag="wpj")
        # small bias loads on gpsimd queue (host pre-transposed, contiguous)
        bqkv_po = persist.tile([P, 18], F32, tag="bqkv")
        nc.gpsimd.dma_start(bqkv_po, bqkv_d[:, :])

        mask_cols = {0: (0, 128), 1: (0, 128), 2: (0, 256), 3: (0, 256),
                     4: (256, 128), 5: (256, 128), 6: (256, 256), 7: (256, 256)}
        masks = {}

        with tc.tile_pool(name="attn_live", bufs=1) as attn_live:
            # packed head-pair K tiles: partitions [0:64]=head 2j, [64:128]=head 2j+1
            kTp = [attn_live.tile([P, T], BF16, tag=f"kTp{j}", name=f"kTp{j}")
                   for j in range(CJ)]
            v_aug = [attn_live.tile([P, 2, H, P], FP8, tag=f"vaug{tp}",
                                    name=f"vaug{tp}") for tp in range(NT // 2)]
            yT = [attn_live.tile([P, TQ], BF16, tag=f"yT{j}", name=f"yT{j}")
                  for j in range(CJ)]
            xlnT = [attn_live.tile([P, 2, T], FP8, tag=f"xlnT{c2}",
                                   name=f"xlnT{c2}") for c2 in range(CJ // 2)]
            # weight tiles resident through the attention block
            vw = attn_live.tile([P, CJ // 2, 2, 768], FP8, tag="vw")
            wkq = [attn_live.tile([P, CJ // 2, 2, 768], FP8, tag=f"wkq{k}",
                                  name=f"wkq{k}") for k in range(2)]
            wo_t = attn_live.tile([P, CJ, C], BF16, tag="wo")

            with tc.tile_pool(name="ph12", bufs=1) as ph12, \
                 tc.tile_pool(name="ps_ln", bufs=2, space="PSUM") as ps_ln, \
                 tc.tile_pool(name="ps_v", bufs=3, space="PSUM") as ps_v:
                # x tiles: own (even permuted positions) f32; odd bf16
                x_sb = []
                for t in range(NT):
                    if t % 2 == 0:
                        x_sb.append(x_own[t // 2])
                    else:
                        xt = ph12.tile([P, C], BF16, tag=f"x{t}", name=f"x{t}")
                        x_sb.append(xt)
                nc.sync.dma_start(x_sb[0], xpe_d[0:P, :])
                nc.sync.dma_start(x_sb[1], xpo_d[0:P, :])
                # mask inputs early on the fast sync queue
                kg_po = ph12.tile([P, NT], F32, tag="kg")
                nc.sync.dma_start(kg_po, kg_d[:, :])
                qg_b = ph12.tile([P, TQ], F32, tag="qgb")
                bcast_dma(nc.sync, qg_b, qg_d, 0, TQ)
                for t in range(2, NT):
                    src = xpe_d if t % 2 == 0 else xpo_d
                    nc.sync.dma_start(x_sb[t], src[(t // 2) * P:(t // 2 + 1) * P, :])
                nc.sync.dma_start(vw, wqkv_d[:, :, 2 * C:3 * C])
                for kp in range(2):
                    nc.sync.dma_start(
                        wkq[kp], wqkv_d[:, :, kp * C:(kp + 1) * C])

                # ---- LN1 + transpose + V projection, pipelined per tile ----
                for t in range(NT):
                    stats = work.tile([P, 2, 6], F32, tag="bnstats")
                    for g in range(2):
                        nc.vector.bn_stats(stats[:, g, :],
                                           x_sb[t][:, g * 384:(g + 1) * 384])
                    mv = work.tile([P, 2], F32, tag="bnmv")
                    nc.vector.bn_aggr(mv, stats)
                    rstd = work.tile([P, 1], F32, tag="rstd")
                    nc.scalar.activation(rstd, mv[:, 1:2], AF.Sqrt, bias=eps_t)
                    nc.vector.reciprocal(rstd, rstd)
                    nmr = work.tile([P, 1], F32, tag="nmr")
                    nc.vector.scalar_tensor_tensor(
                        nmr, mv[:, 0:1], -1.0, rstd, ALU.mult, ALU.mult)
                    xln = work.tile([P, C], F32R, tag="xln")
                    nc.scalar.activation(xln, x_sb[t], AF.Identity,
                                         bias=nmr, scale=rstd)
                    for j in range(CJ):
                        ptr = ps_ln.tile([P, P], F32R, tag="tr")
                        nc.tensor.transpose(ptr, xln[:, j * P:(j + 1) * P],
                                            ident_r)
                        dst = xlnT[j // 2][:, j % 2, t * P:(t + 1) * P]
                        if j in (0, 3):
                            nc.vector.tensor_scalar(dst, ptr, XS, None,
                                                    ALU.mult)
                        else:
                            nc.scalar.activation(dst, ptr, AF.Identity,
                                                 scale=XS)
                    # V for this token tile (V bias folded into bo host-side)
                    pmv = [ps_v.tile([P, 384], F32, tag="mmv", name=f"pmv{_h}")
                           for _h in range(2)]
                    for c2 in range(CJ // 2):
                        for half in range(2):
                            nc.tensor.matmul(
                                pmv[half],
                                xlnT[c2][:, :, t * P:(t + 1) * P],
                                vw[:, c2, :, half * 384:(half + 1) * 384],
                                start=(c2 == 0), stop=(c2 == CJ // 2 - 1),
                                perf_mode=DR, skip_group_check=True)
                    nc.vector.tensor_scalar(
                        v_aug[t // 2][:, t % 2, 0:6, 0:HD],
                        pmv[0].rearrange("p (h d) -> p h d", d=HD),
                        1.0 / (WS * XS), None, ALU.mult)
                    nc.scalar.activation(
                        v_aug[t // 2][:, t % 2, 6:12, 0:HD],
                        pmv[1].rearrange("p (h d) -> p h d", d=HD),
                        AF.Identity, scale=1.0 / (WS * XS))

                # masks + v_aug ones column (before first AV use)
                for kc in range(NT):
                    off, w = mask_cols[kc]
                    m = persist.tile([P, w], FP8, tag=f"mask{kc}",
                                     name=f"mask{kc}")
                    nc.vector.tensor_scalar(
                        m, qg_b[:, off:off + w], kg_po[:, kc:kc + 1], None,
                        ALU.is_ge)
                    masks[kc] = m
                for tp in range(NT // 2):
                    nc.vector.tensor_copy(
                        v_aug[tp][:, :, :, HD:HD + 1],
                        ones_col_f32.to_broadcast([P, 2, H, 1]))
                    nc.vector.memset(v_aug[tp][:, :, :, HD + 1:HD + 2], 0.0)

            # ---- K, Q (pipelined one tile ahead), attention per tile j ----
            with tc.tile_pool(name="att", bufs=3) as att, \
                 tc.tile_pool(name="ps_kq", bufs=3, space="PSUM") as ps_kq, \
                 tc.tile_pool(name="ps_sc", bufs=2, space="PSUM") as ps_sc, \
                 tc.tile_pool(name="ps_av", bufs=2, space="PSUM") as ps_av, \
                 tc.tile_pool(name="ps_bc", bufs=1, space="PSUM") as ps_bc:

                def kq_tile(j):
                    kp, jl = divmod(j, 3)
                    # K: both halves, c2-outer for stationary reuse
                    pmk = [ps_kq.tile([P, 512], F32, tag="kq",
                                      name=f"pmk{_h}") for _h in range(2)]
                    for c2 in range(CJ // 2):
                        for half in range(2):
                            nc.tensor.matmul(
                                pmk[half],
                                wkq[kp][:, c2, :, jl * P:(jl + 1) * P],
                                xlnT[c2][:, :, half * 512:(half + 1) * 512],
                                start=(c2 == 0), stop=(c2 == CJ // 2 - 1),
                                perf_mode=DR, skip_group_check=True)
                    for half in range(2):
                        nc.scalar.activation(
                            kTp[j][:, half * 512:(half + 1) * 512],
                            pmk[half], AF.Identity, scale=1.0 / (WS * XS),
                            bias=bqkv_po[:, CJ + j:CJ + j + 1])
                    # Q: own (even) blocks only; per-head zero-padded tiles
                    pmq = ps_kq.tile([P, 512], F32, tag="kq")
                    for c2 in range(CJ // 2):
                        own = xlnT[c2].rearrange(
                            "p i (b c) -> p i b c", c=P)[:, :, 0::2, :]
                        nc.tensor.matmul(
                            pmq,
                            wkq[kp][:, c2, :, 384 + jl * P:384 + (jl + 1) * P],
                            own,
                            start=(c2 == 0), stop=(c2 == CJ // 2 - 1),
                            perf_mode=DR)
                    for hh in range(2):
                        hs = slice(hh * HD, (hh + 1) * HD)
                        nc.scalar.activation(
                            qT[2 * j + hh][hs, :], pmq[hs, :], AF.Identity,
                            scale=1.0 / (WS * XS),
                            bias=bqkv_po[hs, j:j + 1])

                kq_tile(0)
                for j in range(CJ):
                    if j + 1 < CJ:
                        kq_tile(j + 1)
                    # ---- attention for the two heads of tile j ----
                    for hh in range(2):
                        h = 2 * j + hh
                        av = ps_av.tile([HD + 2, 512], F32, tag="av")
                        prev = None
                        for pr in range(NT // 2):
                            n0 = 0 if pr < 2 else 256
                            w = 512 - n0
                            exp = att.tile([P, 2, 512], FP8, tag="exp")
                            for i in range(2):
                                kc = 2 * pr + i
                                sc = ps_sc.tile([P, 512], F32, tag="sc")
                                nc.tensor.matmul(
                                    sc[:, 0:w],
                                    kTp[j][:, kc * P:(kc + 1) * P],
                                    qT[h][:, n0:512],
                                    start=True, stop=True)
                                nc.scalar.activation(exp[:, i, 0:w],
                                                     sc[:, 0:w],
                                                     AF.Exp, scale=0.125)
                                off, wm = mask_cols[kc]
                                loc = off - n0
                                nc.gpsimd.tensor_tensor(
                                    exp[:, i, loc:loc + wm],
                                    exp[:, i, loc:loc + wm],
                                    masks[kc], ALU.mult)
                            if prev is not None:
                                ppr, pn0, pw, pexp = prev
                                nc.tensor.matmul(
                                    av[:, pn0:512],
                                    v_aug[ppr][:, :, h, 0:HD + 2],
                                    pexp[:, :, 0:pw],
                                    start=(ppr == 0), stop=False,
                                    perf_mode=DR, skip_group_check=True)
                            prev = (pr, n0, w, exp)
                        ppr, pn0, pw, pexp = prev
                        nc.tensor.matmul(
                            av[:, pn0:512], v_aug[ppr][:, :, h, 0:HD + 2],
                            pexp[:, :, 0:pw],
                            start=False, stop=True,
                            perf_mode=DR, skip_group_check=True)
                        sums_bf = att.tile([1, 512], BF16, tag="sums")
                        nc.vector.tensor_copy(sums_bf, av[HD:HD + 1, :])
                        bc = ps_bc.tile([HD, 512], F32, tag="bc")
                        nc.tensor.matmul(bc, ones1, sums_bf,
                                         start=True, stop=True)
                        rb = att.tile([HD, 512], F32, tag="rb")
                        with nc.allow_low_precision(reason="softmax denom"):
                            nc.vector.reciprocal_approx_fast(rb, bc)
                        hs = slice(hh * HD, (hh + 1) * HD)
                        nc.vector.tensor_tensor(
                            yT[j][hs, :], av[0:HD, :], rb, ALU.mult)

            # ---- phase 4: x_own^T + Wo -> x1T, LN2 stats inline ----
            nc.sync.dma_start(
                wo_t, wo_d[:, :].rearrange("(o p) n -> p o n", p=P))
            nc.sync.dma_start(wpj_t, wproj_d[:, :, :])
            bo_po = persist.tile([P, CJ], F32, tag="bo")
            nc.gpsimd.dma_start(bo_po, bo_d[:, :])
            bfc_po = persist.tile([P, FCJ], F32, tag="bfc")
            nc.gpsimd.dma_start(bfc_po, bfc_d[:, :])
            bproj_po = persist.tile([P, CJ], F32, tag="bproj")
            nc.gpsimd.dma_start(bproj_po, bproj_d[:, :])

            with tc.tile_pool(name="ph4", bufs=3) as ph4, \
                 tc.tile_pool(name="ps_mm4", bufs=2, space="PSUM") as ps_mm4, \
                 tc.tile_pool(name="ps_st", bufs=1, space="PSUM") as ps_st, \
                 tc.tile_pool(name="ps_bc2", bufs=1, space="PSUM") as ps_bc2:
                mu_ps = ps_st.tile([1, TQ], F32, tag="mups", name="mups")
                sq_ps = ps_st.tile([1, TQ], F32, tag="sqps", name="sqps")
                for m in range(CJ):
                    pm = ps_mm4.tile([P, TQ], F32, tag="mm")
                    for t in range(NQT):
                        nc.tensor.matmul(
                            pm[:, t * P:(t + 1) * P],
                            x_own[t][:, m * P:(m + 1) * P], ident,
                            is_transpose=True,
                            start=(t == 0), stop=False,
                            skip_group_check=True)
                    for kc in range(CJ):
                        nc.tensor.matmul(
                            pm, wo_t[:, kc, m * P:(m + 1) * P], yT[kc],
                            start=False, stop=(kc == CJ - 1),
                            skip_group_check=True)
                    with nc.allow_low_precision(reason="residual f32r"):
                        nc.scalar.activation(x1T[m], pm, AF.Identity,
                                             bias=bo_po[:, m:m + 1])
                    nc.tensor.matmul(mu_ps, ones_col_r, x1T[m],
                                     start=(m == 0), stop=(m == CJ - 1))
                    sq = ph4.tile([P, TQ], F32R, tag="sq")
                    nc.scalar.activation(sq, x1T[m], AF.Square)
                    nc.tensor.matmul(sq_ps, ones_col_r, sq,
                                     start=(m == 0), stop=(m == CJ - 1))

                # LN2 scalars: var = sq/C - (mu/C)^2; rstd = 1/sqrt(var+eps)
                mu_s = ln2c_p.tile([1, TQ], F32, tag="mus")
                nc.vector.tensor_scalar(mu_s, mu_ps, 1.0 / C, None, ALU.mult)
                musq = ln2c_p.tile([1, TQ], F32, tag="musq")
                nc.vector.tensor_tensor(musq, mu_s, mu_s, ALU.mult)
                var_f = ln2c_p.tile([1, TQ], F32, tag="varf")
                nc.vector.scalar_tensor_tensor(
                    var_f, sq_ps, 1.0 / C, musq, ALU.mult, ALU.subtract)
                rstd_f = ln2c_p.tile([1, TQ], F32, tag="rstdf")
                nc.scalar.activation(rstd_f, var_f, AF.Sqrt, bias=eps_t[0:1, :])
                nc.vector.reciprocal(rstd_f, rstd_f)
                # scaled (xXS) bf16 rows for broadcast
                rstd_bf = ln2c_p.tile([1, TQ], BF16, tag="rstdbf")
                nc.vector.tensor_scalar(rstd_bf, rstd_f, XS, None, ALU.mult)
                murstd_bf = ln2c_p.tile([1, TQ], BF16, tag="murstdbf")
                nc.vector.scalar_tensor_tensor(
                    murstd_bf, mu_s, XS, rstd_f, ALU.mult, ALU.mult)
                rstd_bc = ps_bc2.tile([P, TQ], F32, tag="rstdbc", name="rstdbc")
                nc.tensor.matmul(rstd_bc, ones_row_bf, rstd_bf,
                                 start=True, stop=True)
                murstd_bc = ps_bc2.tile([P, TQ], F32, tag="murstdbc",
                                        name="murstdbc")
                nc.tensor.matmul(murstd_bc, ones_row_bf, murstd_bf,
                                 start=True, stop=True)
                # pre-normalized fp8 x1 (x XS), paired for DoubleRow
                x1p = [ln2c_p.tile([P, 2, TQ], FP8, tag=f"x1p{c}",
                                   name=f"x1p{c}") for c in range(CJ // 2)]
                for m in range(CJ):
                    tmp = ph4.tile([P, TQ], F32, tag="x1s")
                    nc.vector.tensor_tensor(tmp, x1T[m], rstd_bc, ALU.mult)
                    nc.vector.tensor_tensor(x1p[m // 2][:, m % 2, :], tmp,
                                            murstd_bc, ALU.subtract)

        # ---------- phases 6-7: FC+gelu, proj+out ----------
        with tc.tile_pool(name="mlp_live", bufs=1) as mlp_live:
            h1p = [mlp_live.tile([P, 2, TQ], FP8, tag=f"h1p{m}",
                                 name=f"h1p{m}") for m in range(FCJ // 2)]

            with tc.tile_pool(name="wfc_p", bufs=3) as wfc_p, \
                 tc.tile_pool(name="ph6", bufs=3) as ph6, \
                 tc.tile_pool(name="ps_mm6", bufs=3, space="PSUM") as ps_mm6:
                for m2 in range(FCJ // 2):
                    if m2 % 2 == 0:
                        wt4 = wfc_p.tile([P, CJ // 2, 2, 512], FP8, tag="wfc")
                        nc.sync.dma_start(
                            wt4, wfc_d[:, :, 2 * m2 * P:(2 * m2 + 4) * P])
                    xb = ph6.tile([P, 2, TQ], F32, tag="xb")
                    sq = ph6.tile([P, 2, TQ], F32, tag="gsq")
                    for i in range(2):
                        m = 2 * m2 + i
                        ml = m % 4
                        pm = ps_mm6.tile([P, TQ], F32, tag="mm")
                        for c2 in range(CJ // 2):
                            nc.tensor.matmul(
                                pm, wt4[:, c2, :, ml * P:(ml + 1) * P],
                                x1p[c2],
                                start=(c2 == 0), stop=(c2 == CJ // 2 - 1),
                                perf_mode=DR, skip_group_check=True)
                        nc.scalar.activation(sq[:, i, :], pm, AF.Square,
                                             scale=1.0 / (WS * XS),
                                             bias=bfc_po[:, m:m + 1])
                        nc.vector.tensor_scalar(xb[:, i, :], pm,
                                                1.0 / (WS * XS),
                                                bfc_po[:, m:m + 1],
                                                ALU.mult, ALU.add)
                    q4 = ph6.tile([P, 2, TQ], F32, tag="q4")
                    nc.gpsimd.tensor_tensor(q4, sq, sq, ALU.mult)
                    u = ph6.tile([P, 2, TQ], F32, tag="u")
                    nc.scalar.activation(u, q4, AF.Sigmoid, scale=2 * GELU_C)
                    nc.vector.scalar_tensor_tensor(
                        h1p[m2][:, :, :], xb, XS, u, ALU.mult, ALU.mult)

            # ---------- phase 7: proj + residual -> out (m-outer) ----------
            with tc.tile_pool(name="ph7", bufs=2) as ph7, \
                 tc.tile_pool(name="out_p", bufs=1) as out_p, \
                 tc.tile_pool(name="ps_pj", bufs=3, space="PSUM") as ps_pj, \
                 tc.tile_pool(name="ps_tr7", bufs=2, space="PSUM") as ps_tr7:
                out_sb = [out_p.tile([P, C], F32, tag=f"osb{t}", name=f"osb{t}")
                          for t in range(NQT)]
                for m in range(CJ):
                    pm = ps_pj.tile([P, TQ], F32, tag="pj")
                    for kc2 in range(FCJ // 2):
                        nc.tensor.matmul(
                            pm, wpj_t[:, kc2, :, m * P:(m + 1) * P], h1p[kc2],
                            start=(kc2 == 0), stop=(kc2 == FCJ // 2 - 1),
                            perf_mode=DR)
                    ojT = ph7.tile([P, TQ], F32R, tag="ojT")
                    nc.vector.tensor_scalar(ojT, pm, 1.0 / (WS * XS),
                                            bproj_po[:, m:m + 1],
                                            ALU.mult, ALU.add)
                    nc.vector.tensor_tensor(ojT, ojT, x1T[m], ALU.add)
                    for t in range(NQT):
                        ptr = ps_tr7.tile([P, P], F32R, tag="tr")
                        nc.tensor.transpose(
                            ptr, ojT[:, t * P:(t + 1) * P], ident_r)
                        dst = out_sb[t][:, m * P:(m + 1) * P]
                        if m % 2 == 0:
                            nc.vector.tensor_copy(dst, ptr)
                        else:
                            nc.scalar.copy(dst, ptr)
                for t in range(NQT):
                    nc.sync.dma_start(out_d[t * P:(t + 1) * P, :], out_sb[t])

    nc.compile()
    return nc


def _get_nc():
    if "nc" not in _CACHED:
        _CACHED["nc"] = _build_nc()
    return _CACHED["nc"]


def _perm_blocks(p):
    return [p, 1 - p, 2 + p, 3 - p, 4 + p, 5 - p, 6 + p, 7 - p]


def _prepare(x, ln1_scale, ln1_bias, Wqkv, bqkv, Wo, bo,
             ln2_scale, ln2_bias, Wfc, bfc, Wproj, bproj):
    """Host-side prep: fold LN params into weights, permute qkv to
    [Q|K|V] layout, pre-transpose bias vectors, build per-core in_maps."""
    FP8NP = ml_dtypes.float8_e4m3
    x = np.asarray(x, np.float32)
    Wqkv64 = np.asarray(Wqkv, np.float64)
    Wqkv64 = np.asarray(ln1_scale, np.float64)[:, None] * Wqkv64
    bqkv64 = np.asarray(bqkv, np.float64) + np.asarray(ln1_bias, np.float64) @ Wqkv64
    Wfc64 = np.asarray(Wfc, np.float64)
    Wfc64 = np.asarray(ln2_scale, np.float64)[:, None] * Wfc64
    bfc64 = np.asarray(bfc, np.float64) + np.asarray(ln2_bias, np.float64) @ Wfc64
    # Reference splits qkv per head: columns are [h0: q|k|v, h1: q|k|v, ...].
    colmap = np.arange(3 * C).reshape(H, 3, HD)
    qkv_perm = np.concatenate(
        [colmap[:, 0, :].ravel(), colmap[:, 1, :].ravel(), colmap[:, 2, :].ravel()])
    Wqkvp = Wqkv64.astype(np.float32)[:, qkv_perm]
    bqkvp = bqkv64.astype(np.float32)[qkv_perm]
    piece_perm = np.concatenate([
        np.arange(C + 0, C + 384),        # K0
        np.arange(0, 384),                # Q0
        np.arange(C + 384, C + 768),      # K1
        np.arange(384, 768),              # Q1
        np.arange(2 * C, 3 * C),          # V
    ])
    Wqkv_dev = Wqkvp[:, piece_perm]
    # fold V bias through Wo into bo (softmax weights sum to 1)
    Wo64 = np.asarray(Wo, np.float64)
    bo64 = np.asarray(bo, np.float64) + bqkv64[qkv_perm][2 * C:] @ Wo64

    def po(v, cols):
        return np.ascontiguousarray(
            np.asarray(v, np.float32).reshape(cols, P).T)

    def w8(w, rows):
        # [rows*P, n] -> [P, rows, n] scaled fp8
        w = np.asarray(w, np.float64) * WS
        return np.ascontiguousarray(
            w.reshape(rows, P, -1).transpose(1, 0, 2).astype(FP8NP))

    shared = {
        "wqkv": w8(Wqkv_dev, CJ),
        "bqkv": po(bqkvp, 18),
        "wo": np.ascontiguousarray(np.asarray(Wo, np.float32).astype(ml_dtypes.bfloat16)),
        "bo": po(bo64.astype(np.float32), CJ),
        "wfc": w8(Wfc64, CJ),
        "bfc": po(bfc64.astype(np.float32), FCJ),
        "wproj": w8(Wproj, FCJ),
        "bproj": po(bproj, CJ),
    }
    in_maps = []
    own_toks = []
    for c in range(N_CORES):
        s, p = divmod(c, 2)
        blocks = _perm_blocks(p)
        tok = np.concatenate([np.arange(b * P, (b + 1) * P) for b in blocks])
        own = np.concatenate([np.arange(b * P, (b + 1) * P) for b in blocks[0::2]])
        odd = np.concatenate([np.arange(b * P, (b + 1) * P) for b in blocks[1::2]])
        own_toks.append((s, own))
        in_maps.append({
            "xpe": np.ascontiguousarray(x[s][own]),
            "xpo": np.ascontiguousarray(x[s][odd].astype(ml_dtypes.bfloat16)),
            "qg": own.astype(np.float32),
            "kg": po(tok.astype(np.float32), NT),
            **shared,
        })
    return in_maps, own_toks


def kernel(x, ln1_scale, ln1_bias, Wqkv, bqkv, Wo, bo,
           ln2_scale, ln2_bias, Wfc, bfc, Wproj, bproj):
    from concourse.bass_utils import run_bass_kernel_spmd

    in_maps, own_toks = _prepare(x, ln1_scale, ln1_bias, Wqkv, bqkv, Wo, bo,
                                 ln2_scale, ln2_bias, Wfc, bfc, Wproj, bproj)
    nc = _get_nc()
    res = run_bass_kernel_spmd(nc, in_maps, list(range(N_CORES)))

    out = np.empty((B, T, C), np.float32)
    for c in range(N_CORES):
        s, own = own_toks[c]
        out[s][own] = res.results[c]["out"]
    return out


# revision 20
# speedup vs baseline: 1.5534x; 1.1231x over previous
"""Trainium2 Bass kernel for a GPT-2-style transformer block.

B=4, T=1024, C=768, H=12 heads (HD=64). 8 NeuronCores.

Sharding: 2 cores per batch sequence. Each core is fed a block-permuted
copy of its sequence (own query blocks at even block positions), computes
K/V for the full sequence locally (no collectives), runs causal attention
for its 512 query tokens with data-driven masks, and the full MLP for
those tokens. Host re-assembles the [B,T,C] output.

QKV/Wo run bf16; FC/proj run fp8e4 DoubleRow (weights x32, acts x16,
rescaled on PSUM read); residual/LN paths stay f32.
"""

import numpy as np
import ml_dtypes

P = 128
B, T, C, H = 4, 1024, 768, 12
HD = C // H        # 64
CJ = C // P        # 6 C-chunks
NT = T // P        # 8 token tiles
TQ = 512           # own query tokens per core
NQT = TQ // P      # 4 q slots
FC = 4 * C         # 3072
FCJ = FC // P      # 24
GELU_C = 0.035677408136300125  # sqrt(2/pi)*0.044715 -> tanh(c*x^4)
WS = 32.0          # fp8 weight scale
XS = 16.0          # fp8 activation scale
N_CORES = 8

_CACHED = {}


def _build_nc():
    import concourse.bass as bass
    from concourse import bacc, mybir
    import concourse.tile as tile
    from concourse.masks import make_identity
    from contextlib import ExitStack

    F32 = mybir.dt.float32
    F32R = mybir.dt.float32r
    BF16 = mybir.dt.bfloat16
    FP8 = mybir.dt.float8e4
    AF = mybir.ActivationFunctionType
    ALU = mybir.AluOpType
    DR = mybir.MatmulPerfMode.DoubleRow

    nc = bacc.Bacc()

    xpe_d = nc.declare_dram_parameter("xpe", [TQ, C], F32, isOutput=False)
    xpo_d = nc.declare_dram_parameter("xpo", [TQ, C], BF16, isOutput=False)
    qg_d = nc.declare_dram_parameter("qg", [TQ], F32, isOutput=False)
    kg_d = nc.declare_dram_parameter("kg", [P, NT], F32, isOutput=False)
    wqkv_d = nc.declare_dram_parameter("wqkv", [P, CJ, 3 * C], FP8, isOutput=False)
    bqkv_d = nc.declare_dram_parameter("bqkv", [P, 18], F32, isOutput=False)
    wo_d = nc.declare_dram_parameter("wo", [C, C], BF16, isOutput=False)
    bo_d = nc.declare_dram_parameter("bo", [P, CJ], F32, isOutput=False)
    wfc_d = nc.declare_dram_parameter("wfc", [P, CJ, FC], FP8, isOutput=False)
    bfc_d = nc.declare_dram_parameter("bfc", [P, FCJ], F32, isOutput=False)
    wproj_d = nc.declare_dram_parameter("wproj", [P, FCJ, C], FP8, isOutput=False)
    bproj_d = nc.declare_dram_parameter("bproj", [P, CJ], F32, isOutput=False)
    out_d = nc.declare_dram_parameter("out", [TQ, C], F32, isOutput=True)

    def bcast_dma(engine, dst, dram_handle, offset, n):
        """DMA [n] DRAM vector broadcast across 128 partitions -> dst[128, n]."""
        ap = dram_handle[:]
        src = bass.AP(tensor=ap.tensor, offset=offset, ap=[[0, P], [1, n]])
        engine.dma_start(dst, src)

    with tile.TileContext(nc) as tc, ExitStack() as ctx:
        persist = ctx.enter_context(tc.tile_pool(name="persist", bufs=1))
        work = ctx.enter_context(tc.tile_pool(name="work", bufs=3))
        ln2c_p = ctx.enter_context(tc.tile_pool(name="ln2c_p", bufs=1))

        # ---------- constants ----------
        ident = persist.tile([P, P], F32, tag="ident")
        make_identity(nc, ident)
        ident_r = persist.tile([P, P], F32R, tag="identr")
        nc.vector.tensor_copy(ident_r, ident)
        eps_t = persist.tile([P, 1], F32, tag="eps")
        nc.vector.memset(eps_t, 1e-5)
        ones1_f32 = persist.tile([1, HD], F32, tag="ones1f")
        nc.vector.memset(ones1_f32, 1.0)
        ones1 = persist.tile([1, HD], BF16, tag="ones1")
        nc.vector.tensor_copy(ones1, ones1_f32)
        ones_col_f32 = persist.tile([P, 1], F32, tag="onescol")
        nc.vector.memset(ones_col_f32, 1.0)
        ones_col_r = persist.tile([P, 1], F32R, tag="onescolr")
        nc.vector.tensor_copy(ones_col_r, ones_col_f32)
        ones_row_bf = persist.tile([1, P], BF16, tag="onesrow")
        nc.vector.tensor_copy(ones_row_bf, ones_col_f32[0:1, 0:1].to_broadcast([1, P]))

        # per-head q tiles (zero-padded other half; K=128 score matmuls)
        qT = [persist.tile([P, TQ], BF16, tag=f"qT{h}", name=f"qT{h}")
              for h in range(H)]
        for h in range(H):
            zlo = slice(0, HD) if h % 2 else slice(HD, P)
            nc.gpsimd.memset(qT[h][zlo, :], 0.0)

        x_own = [persist.tile([P, C], F32, tag=f"xo{t}", name=f"xo{t}")
                 for t in range(NQT)]
        x1T = [persist.tile([P, TQ], F32R, tag=f"x1T{m}", name=f"x1T{m}")
               for m in range(CJ)]

        wpj_t = persist.tile([P, FCJ // 2, 2, C], FP8, tag="wpj")
        # small bias loads on gpsimd queue (host pre-transposed, contiguous)
        bqkv_po = persist.tile([P, 18], F32, tag="bqkv")
        nc.gpsimd.dma_start(bqkv_po, bqkv_d[:, :])

        mask_cols = {0: (0, 128), 1: (0, 128), 2: (0, 256), 3: (0, 256),
                     4: (256, 128), 5: (256, 128), 6: (256, 256), 7: (256, 256)}
        masks = {}

        with tc.tile_pool(name="attn_live", bufs=1) as attn_live:
            # packed head-pair K tiles: partitions [0:64]=head 2j, [64:128]=head 2j+1
            kTp = [attn_live.tile([P, T], BF16, tag=f"kTp{j}", name=f"kTp{j}")
                   for j in range(CJ)]
            v_aug = [attn_live.tile([P, 2, H, P], FP8, tag=f"vaug{tp}",
                                    name=f"vaug{tp}") for tp in range(NT // 2)]
            yT = [attn_live.tile([P, TQ], BF16, tag=f"yT{j}", name=f"yT{j}")
                  for j in range(CJ)]
            xlnT = [attn_live.tile([P, 2, T], FP8, tag=f"xlnT{c2}",
                                   name=f"xlnT{c2}") for c2 in range(CJ // 2)]
            # weight tiles resident through the attention block
            vw = attn_live.tile([P, CJ // 2, 2, 768], FP8, tag="vw")
            wkq = [attn_live.tile([P, CJ // 2, 2, 768], FP8, tag=f"wkq{k}",
                                  name=f"wkq{k}") for k in range(2)]
            wo_t = attn_live.tile([P, CJ, C], BF16, tag="wo")

            with tc.tile_pool(name="ph12", bufs=1) as ph12, \
                 tc.tile_pool(name="ps_ln", bufs=2, space="PSUM") as ps_ln, \
                 tc.tile_pool(name="ps_v", bufs=3, space="PSUM") as ps_v:
                # x tiles: own (even permuted positions) f32; odd bf16
                x_sb = []
                for t in range(NT):
                    if t % 2 == 0:
                        x_sb.append(x_own[t // 2])
                    else:
                        xt = ph12.tile([P, C], BF16, tag=f"x{t}", name=f"x{t}")
                        x_sb.append(xt)
                nc.sync.dma_start(x_sb[0], xpe_d[0:P, :])
                nc.sync.dma_start(x_sb[1], xpo_d[0:P, :])
                # mask inputs early on the fast sync queue
                kg_po = ph12.tile([P, NT], F32, tag="kg")
                nc.sync.dma_start(kg_po, kg_d[:, :])
                qg_b = ph12.tile([P, TQ], F32, tag="qgb")
                bcast_dma(nc.sync, qg_b, qg_d, 0, TQ)
                for t in range(2, NT):
                    src = xpe_d if t % 2 == 0 else xpo_d
                    nc.sync.dma_start(x_sb[t], src[(t // 2) * P:(t // 2 + 1) * P, :])
                nc.sync.dma_start(vw, wqkv_d[:, :, 2 * C:3 * C])
                for kp in range(2):
                    nc.sync.dma_start(
                        wkq[kp], wqkv_d[:, :, kp * C:(kp + 1) * C])

                # ---- LN1 + transpose + V projection, pipelined per tile ----
                for t in range(NT):
                    stats = work.tile([P, 2, 6], F32, tag="bnstats")
                    for g in range(2):
                        nc.vector.bn_stats(stats[:, g, :],
                                           x_sb[t][:, g * 384:(g + 1) * 384])
                    mv = work.tile([P, 2], F32, tag="bnmv")
                    nc.vector.bn_aggr(mv, stats)
                    rstd = work.tile([P, 1], F32, tag="rstd")
                    nc.scalar.activation(rstd, mv[:, 1:2], AF.Sqrt, bias=eps_t)
                    nc.vector.reciprocal(rstd, rstd)
                    nmr = work.tile([P, 1], F32, tag="nmr")
                    nc.vector.scalar_tensor_tensor(
                        nmr, mv[:, 0:1], -1.0, rstd, ALU.mult, ALU.mult)
                    xln = work.tile([P, C], F32R, tag="xln")
                    nc.scalar.activation(xln, x_sb[t], AF.Identity,
                                         bias=nmr, scale=rstd)
                    for j in range(CJ):
                        ptr = ps_ln.tile([P, P], F32R, tag="tr")
                        nc.tensor.transpose(ptr, xln[:, j * P:(j + 1) * P],
                                            ident_r)
                        dst = xlnT[j // 2][:, j % 2, t * P:(t + 1) * P]
                        if j in (0, 2, 4):
                            nc.vector.tensor_scalar(dst, ptr, XS, None,
                                                    ALU.mult)
                        else:
                            nc.scalar.activation(dst, ptr, AF.Identity,
                                                 scale=XS)
                    # V for this token tile (V bias folded into bo host-side)
                    pmv = [ps_v.tile([P, 384], F32, tag="mmv", name=f"pmv{_h}")
                           for _h in range(2)]
                    for c2 in range(CJ // 2):
                        for half in range(2):
                            nc.tensor.matmul(
                                pmv[half],
                                xlnT[c2][:, :, t * P:(t + 1) * P],
                                vw[:, c2, :, half * 384:(half + 1) * 384],
                                start=(c2 == 0), stop=(c2 == CJ // 2 - 1),
                                perf_mode=DR, skip_group_check=True)
                    nc.vector.tensor_scalar(
                        v_aug[t // 2][:, t % 2, 0:6, 0:HD],
                        pmv[0].rearrange("p (h d) -> p h d", d=HD),
                        1.0 / (WS * XS), None, ALU.mult)
                    nc.scalar.activation(
                        v_aug[t // 2][:, t % 2, 6:12, 0:HD],
                        pmv[1].rearrange("p (h d) -> p h d", d=HD),
                        AF.Identity, scale=1.0 / (WS * XS))

                # masks + v_aug ones column (before first AV use)
                for kc in range(NT):
                    off, w = mask_cols[kc]
                    m = persist.tile([P, w], FP8, tag=f"mask{kc}",
                                     name=f"mask{kc}")
                    nc.vector.tensor_scalar(
                        m, qg_b[:, off:off + w], kg_po[:, kc:kc + 1], None,
                        ALU.is_ge)
                    masks[kc] = m
                for tp in range(NT // 2):
                    nc.vector.tensor_copy(
                        v_aug[tp][:, :, :, HD:HD + 1],
                        ones_col_f32.to_broadcast([P, 2, H, 1]))
                    nc.vector.memset(v_aug[tp][:, :, :, HD + 1:HD + 2], 0.0)

            # ---- K, Q (pipelined one tile ahead), attention per tile j ----
            with tc.tile_pool(name="att", bufs=3) as att, \
                 tc.tile_pool(name="ps_kq", bufs=3, space="PSUM") as ps_kq, \
                 tc.tile_pool(name="ps_sc", bufs=2, space="PSUM") as ps_sc, \
                 tc.tile_pool(name="ps_av", bufs=2, space="PSUM") as ps_av, \
                 tc.tile_pool(name="ps_bc", bufs=1, space="PSUM") as ps_bc:

                def kq_tile(j):
                    kp, jl = divmod(j, 3)
                    # K: both halves, c2-outer for stationary reuse
                    pmk = [ps_kq.tile([P, 512], F32, tag="kq",
                                      name=f"pmk{_h}") for _h in range(2)]
                    for c2 in range(CJ // 2):
                        for half in range(2):
                            nc.tensor.matmul(
                                pmk[half],
                                wkq[kp][:, c2, :, jl * P:(jl + 1) * P],
                                xlnT[c2][:, :, half * 512:(half + 1) * 512],
                                start=(c2 == 0), stop=(c2 == CJ // 2 - 1),
                                perf_mode=DR, skip_group_check=True)
                    for half in range(2):
                        nc.vector.tensor_scalar(
                            kTp[j][:, half * 512:(half + 1) * 512],
                            pmk[half], 1.0 / (WS * XS),
                            bqkv_po[:, CJ + j:CJ + j + 1],
                            ALU.mult, ALU.add)
                    # Q: own (even) blocks only; per-head zero-padded tiles
                    pmq = ps_kq.tile([P, 512], F32, tag="kq")
                    for c2 in range(CJ // 2):
                        own = xlnT[c2].rearrange(
                            "p i (b c) -> p i b c", c=P)[:, :, 0::2, :]
                        nc.tensor.matmul(
                            pmq,
                            wkq[kp][:, c2, :, 384 + jl * P:384 + (jl + 1) * P],
                            own,
                            start=(c2 == 0), stop=(c2 == CJ // 2 - 1),
                            perf_mode=DR)
                    for hh in range(2):
                        hs = slice(hh * HD, (hh + 1) * HD)
                        nc.vector.tensor_scalar(
                            qT[2 * j + hh][hs, :], pmq[hs, :],
                            1.0 / (WS * XS), bqkv_po[hs, j:j + 1],
                            ALU.mult, ALU.add)

                kq_tile(0)
                for j in range(CJ):
                    if j + 1 < CJ:
                        kq_tile(j + 1)
                    # ---- attention for the two heads of tile j ----
                    for hh in range(2):
                        h = 2 * j + hh
                        av = ps_av.tile([HD + 2, 512], F32, tag="av")
                        prev = None
                        for pr in range(NT // 2):
                            n0 = 0 if pr < 2 else 256
                            w = 512 - n0
                            exp = att.tile([P, 2, 512], FP8, tag="exp")
                            if pr >= 2:
                                sc = ps_sc.tile([P, 512], F32, tag="sc")
                                for i in range(2):
                                    kc = 2 * pr + i
                                    nc.tensor.matmul(
                                        sc[:, i * 256:(i + 1) * 256],
                                        kTp[j][:, kc * P:(kc + 1) * P],
                                        qT[h][:, n0:512],
                                        start=True, stop=True,
                                        skip_group_check=True)
                                nc.scalar.activation(
                                    exp[:, :, 0:256], sc,
                                    AF.Exp, scale=0.125)
                            else:
                                for i in range(2):
                                    kc = 2 * pr + i
                                    sc = ps_sc.tile([P, 512], F32, tag="sc")
                                    nc.tensor.matmul(
                                        sc[:, 0:w],
                                        kTp[j][:, kc * P:(kc + 1) * P],
                                        qT[h][:, n0:512],
                                        start=True, stop=True)
                                    nc.scalar.activation(exp[:, i, 0:w],
                                                         sc[:, 0:w],
                                                         AF.Exp, scale=0.125)
                            for i in range(2):
                                kc = 2 * pr + i
                                off, wm = mask_cols[kc]
                                loc = off - n0
                                nc.gpsimd.tensor_tensor(
                                    exp[:, i, loc:loc + wm],
                                    exp[:, i, loc:loc + wm],
                                    masks[kc], ALU.mult)
                            if prev is not None:
                                ppr, pn0, pw, pexp = prev
                                nc.tensor.matmul(
                                    av[:, pn0:512],
                                    v_aug[ppr][:, :, h, 0:HD + 2],
                                    pexp[:, :, 0:pw],
                                    start=(ppr == 0), stop=False,
                                    perf_mode=DR, skip_group_check=True)
                            prev = (pr, n0, w, exp)
                        ppr, pn0, pw, pexp = prev
                        nc.tensor.matmul(
                            av[:, pn0:512], v_aug[ppr][:, :, h, 0:HD + 2],
                            pexp[:, :, 0:pw],
                            start=False, stop=True,
                            perf_mode=DR, skip_group_check=True)
                        sums_bf = att.tile([1, 512], BF16, tag="sums")
                        nc.vector.tensor_copy(sums_bf, av[HD:HD + 1, :])
                        bc = ps_bc.tile([HD, 512], F32, tag="bc")
                        nc.tensor.matmul(bc, ones1, sums_bf,
                                         start=True, stop=True)
                        rb = att.tile([HD, 512], F32, tag="rb")
                        with nc.allow_low_precision(reason="softmax denom"):
                            nc.vector.reciprocal_approx_fast(rb, bc)
                        hs = slice(hh * HD, (hh + 1) * HD)
                        nc.vector.tensor_tensor(
                            yT[j][hs, :], av[0:HD, :], rb, ALU.mult)

            # ---- phase 4: x_own^T + Wo -> x1T, LN2 stats inline ----
            nc.sync.dma_start(
                wo_t, wo_d[:, :].rearrange("(o p) n -> p o n", p=P))
            nc.sync.dma_start(wpj_t, wproj_d[:, :, :])
            bo_po = persist.tile([P, CJ], F32, tag="bo")
            nc.gpsimd.dma_start(bo_po, bo_d[:, :])
            bfc_po = persist.tile([P, FCJ], F32, tag="bfc")
            nc.gpsimd.dma_start(bfc_po, bfc_d[:, :])
            bproj_po = persist.tile([P, CJ], F32, tag="bproj")
            nc.gpsimd.dma_start(bproj_po, bproj_d[:, :])

            with tc.tile_pool(name="ph4", bufs=3) as ph4, \
                 tc.tile_pool(name="ps_mm4", bufs=2, space="PSUM") as ps_mm4, \
                 tc.tile_pool(name="ps_st", bufs=1, space="PSUM") as ps_st, \
                 tc.tile_pool(name="ps_bc2", bufs=1, space="PSUM") as ps_bc2:
                mu_ps = ps_st.tile([1, TQ], F32, tag="mups", name="mups")
                sq_ps = ps_st.tile([1, TQ], F32, tag="sqps", name="sqps")
                for m in range(CJ):
                    pm = ps_mm4.tile([P, TQ], F32, tag="mm")
                    for t in range(NQT):
                        nc.tensor.matmul(
                            pm[:, t * P:(t + 1) * P],
                            x_own[t][:, m * P:(m + 1) * P], ident,
                            is_transpose=True,
                            start=(t == 0), stop=False,
                            skip_group_check=True)
                    for kc in range(CJ):
                        nc.tensor.matmul(
                            pm, wo_t[:, kc, m * P:(m + 1) * P], yT[kc],
                            start=False, stop=(kc == CJ - 1),
                            skip_group_check=True)
                    with nc.allow_low_precision(reason="residual f32r"):
                        nc.vector.tensor_scalar(x1T[m], pm,
                                                bo_po[:, m:m + 1], None,
                                                ALU.add)
                    nc.tensor.matmul(mu_ps, ones_col_r, x1T[m],
                                     start=(m == 0), stop=(m == CJ - 1))
                    sq = ph4.tile([P, TQ], F32R, tag="sq")
                    nc.scalar.activation(sq, x1T[m], AF.Square)
                    nc.tensor.matmul(sq_ps, ones_col_r, sq,
                                     start=(m == 0), stop=(m == CJ - 1))

                # LN2 scalars: var = sq/C - (mu/C)^2; rstd = 1/sqrt(var+eps)
                mu_s = ln2c_p.tile([1, TQ], F32, tag="mus")
                nc.vector.tensor_scalar(mu_s, mu_ps, 1.0 / C, None, ALU.mult)
                musq = ln2c_p.tile([1, TQ], F32, tag="musq")
                nc.vector.tensor_tensor(musq, mu_s, mu_s, ALU.mult)
                var_f = ln2c_p.tile([1, TQ], F32, tag="varf")
                nc.vector.scalar_tensor_tensor(
                    var_f, sq_ps, 1.0 / C, musq, ALU.mult, ALU.subtract)
                rstd_f = ln2c_p.tile([1, TQ], F32, tag="rstdf")
                nc.scalar.activation(rstd_f, var_f, AF.Sqrt, bias=eps_t[0:1, :])
                nc.vector.reciprocal(rstd_f, rstd_f)
                # scaled (xXS) bf16 rows for broadcast
                rstd_bf = ln2c_p.tile([1, TQ], BF16, tag="rstdbf")
                nc.vector.tensor_scalar(rstd_bf, rstd_f, XS, None, ALU.mult)
                murstd_bf = ln2c_p.tile([1, TQ], BF16, tag="murstdbf")
                nc.vector.scalar_tensor_tensor(
                    murstd_bf, mu_s, XS, rstd_f, ALU.mult, ALU.mult)
                rstd_bc = ps_bc2.tile([P, TQ], F32, tag="rstdbc", name="rstdbc")
                nc.tensor.matmul(rstd_bc, ones_row_bf, rstd_bf,
                                 start=True, stop=True)
                murstd_bc = ps_bc2.tile([P, TQ], F32, tag="murstdbc",
                                        name="murstdbc")
                nc.tensor.matmul(murstd_bc, ones_row_bf, murstd_bf,
                                 start=True, stop=True)
                # pre-normalized fp8 x1 (x XS), paired for DoubleRow
                x1p = [ln2c_p.tile([P, 2, TQ], FP8, tag=f"x1p{c}",
                                   name=f"x1p{c}") for c in range(CJ // 2)]
                for m in range(CJ):
                    tmp = ph4.tile([P, TQ], F32, tag="x1s")
                    nc.vector.tensor_tensor(tmp, x1T[m], rstd_bc, ALU.mult)
                    nc.vector.tensor_tensor(x1p[m // 2][:, m % 2, :], tmp,
                                            murstd_bc, ALU.subtract)

        # ---------- phases 6-7: FC+gelu, proj+out ----------
        with tc.tile_pool(name="mlp_live", bufs=1) as mlp_live:
            h1p = [mlp_live.tile([P, 2, TQ], FP8, tag=f"h1p{m}",
                                 name=f"h1p{m}") for m in range(FCJ // 2)]

            with tc.tile_pool(name="wfc_p", bufs=3) as wfc_p, \
                 tc.tile_pool(name="ph6", bufs=3) as ph6, \
                 tc.tile_pool(name="ps_mm6", bufs=4, space="PSUM") as ps_mm6:
                for m2 in range(FCJ // 2):
                    if m2 % 2 == 0:
                        wt4 = wfc_p.tile([P, CJ // 2, 2, 512], FP8, tag="wfc")
                        nc.sync.dma_start(
                            wt4, wfc_d[:, :, 2 * m2 * P:(2 * m2 + 4) * P])
                    xb = ph6.tile([P, 2, TQ], F32, tag="xb")
                    sq = ph6.tile([P, 2, TQ], F32, tag="gsq")
                    for i in range(2):
                        m = 2 * m2 + i
                        ml = m % 4
                        pm = ps_mm6.tile([P, TQ], F32, tag="mm")
                        for c2 in range(CJ // 2):
                            nc.tensor.matmul(
                                pm, wt4[:, c2, :, ml * P:(ml + 1) * P],
                                x1p[c2],
                                start=(c2 == 0), stop=(c2 == CJ // 2 - 1),
                                perf_mode=DR, skip_group_check=True)
                        nc.scalar.activation(sq[:, i, :], pm, AF.Square,
                                             scale=1.0 / (WS * XS),
                                             bias=bfc_po[:, m:m + 1])
                        nc.vector.tensor_scalar(xb[:, i, :], pm,
                                                1.0 / (WS * XS),
                                                bfc_po[:, m:m + 1],
                                                ALU.mult, ALU.add)
                    q4 = ph6.tile([P, 2, TQ], F32, tag="q4")
                    nc.gpsimd.tensor_tensor(q4, sq, sq, ALU.mult)
                    u = ph6.tile([P, 2, TQ], F32, tag="u")
                    nc.scalar.activation(u, q4, AF.Sigmoid, scale=2 * GELU_C)
                    nc.vector.scalar_tensor_tensor(
                        h1p[m2][:, :, :], xb, XS, u, ALU.mult, ALU.mult)

            # ---------- phase 7: proj + residual -> out (m-outer) ----------
            with tc.tile_pool(name="ph7", bufs=2) as ph7, \
                 tc.tile_pool(name="out_p", bufs=1) as out_p, \
                 tc.tile_pool(name="ps_pj", bufs=3, space="PSUM") as ps_pj, \
                 tc.tile_pool(name="ps_tr7", bufs=4, space="PSUM") as ps_tr7:
                out_sb = [out_p.tile([P, C], F32, tag=f"osb{t}", name=f"osb{t}")
                          for t in range(NQT)]
                for m in range(CJ):
                    pm = ps_pj.tile([P, TQ], F32, tag="pj")
                    for kc2 in range(FCJ // 2):
                        nc.tensor.matmul(
                            pm, wpj_t[:, kc2, :, m * P:(m + 1) * P], h1p[kc2],
                            start=(kc2 == 0), stop=(kc2 == FCJ // 2 - 1),
                            perf_mode=DR)
                    ojT = ph7.tile([P, TQ], F32, tag="ojT")
                    nc.vector.tensor_scalar(ojT, pm, 1.0 / (WS * XS),
                                            bproj_po[:, m:m + 1],
                                            ALU.mult, ALU.add)
                    nc.vector.tensor_tensor(ojT, ojT, x1T[m], ALU.add)
                    for t in range(NQT):
                        ptr = ps_tr7.tile([P, P], F32, tag="tr")
                        nc.tensor.transpose(
                            ptr, ojT[:, t * P:(t + 1) * P], ident)
                        dst = out_sb[t][:, m * P:(m + 1) * P]
                        if m % 2 == 0:
                            nc.vector.tensor_copy(dst, ptr)
                        else:
                            nc.scalar.copy(dst, ptr)
                        nc.sync.dma_start(
                            out_d[t * P:(t + 1) * P, m * P:(m + 1) * P], dst)

    nc.compile()
    return nc


def _get_nc():
    if "nc" not in _CACHED:
        _CACHED["nc"] = _build_nc()
    return _CACHED["nc"]


def _perm_blocks(p):
    return [p, 1 - p, 2 + p, 3 - p, 4 + p, 5 - p, 6 + p, 7 - p]


def _prepare(x, ln1_scale, ln1_bias, Wqkv, bqkv, Wo, bo,
             ln2_scale, ln2_bias, Wfc, bfc, Wproj, bproj):
    """Host-side prep: fold LN params into weights, permute qkv to
    [Q|K|V] layout, pre-transpose bias vectors, build per-core in_maps."""
    FP8NP = ml_dtypes.float8_e4m3
    x = np.asarray(x, np.float32)
    Wqkv64 = np.asarray(Wqkv, np.float64)
    Wqkv64 = np.asarray(ln1_scale, np.float64)[:, None] * Wqkv64
    bqkv64 = np.asarray(bqkv, np.float64) + np.asarray(ln1_bias, np.float64) @ Wqkv64
    Wfc64 = np.asarray(Wfc, np.float64)
    Wfc64 = np.asarray(ln2_scale, np.float64)[:, None] * Wfc64
    bfc64 = np.asarray(bfc, np.float64) + np.asarray(ln2_bias, np.float64) @ Wfc64
    # Reference splits qkv per head: columns are [h0: q|k|v, h1: q|k|v, ...].
    colmap = np.arange(3 * C).reshape(H, 3, HD)
    qkv_perm = np.concatenate(
        [colmap[:, 0, :].ravel(), colmap[:, 1, :].ravel(), colmap[:, 2, :].ravel()])
    Wqkvp = Wqkv64.astype(np.float32)[:, qkv_perm]
    bqkvp = bqkv64.astype(np.float32)[qkv_perm]
    piece_perm = np.concatenate([
        np.arange(C + 0, C + 384),        # K0
        np.arange(0, 384),                # Q0
        np.arange(C + 384, C + 768),      # K1
        np.arange(384, 768),              # Q1
        np.arange(2 * C, 3 * C),          # V
    ])
    Wqkv_dev = Wqkvp[:, piece_perm]
    # fold V bias through Wo into bo (softmax weights sum to 1)
    Wo64 = np.asarray(Wo, np.float64)
    bo64 = np.asarray(bo, np.float64) + bqkv64[qkv_perm][2 * C:] @ Wo64

    def po(v, cols):
        return np.ascontiguousarray(
            np.asarray(v, np.float32).reshape(cols, P).T)

    def w8(w, rows):
        # [rows*P, n] -> [P, rows, n] scaled fp8
        w = np.asarray(w, np.float64) * WS
        return np.ascontiguousarray(
            w.reshape(rows, P, -1).transpose(1, 0, 2).astype(FP8NP))

    shared = {
        "wqkv": w8(Wqkv_dev, CJ),
        "bqkv": po(bqkvp, 18),
        "wo": np.ascontiguousarray(np.asarray(Wo, np.float32).astype(ml_dtypes.bfloat16)),
        "bo": po(bo64.astype(np.float32), CJ),
        "wfc": w8(Wfc64, CJ),
        "bfc": po(bfc64.astype(np.float32), FCJ),
        "wproj": w8(Wproj, FCJ),
        "bproj": po(bproj, CJ),
    }
    in_maps = []
    own_toks = []
    for c in range(N_CORES):
        s, p = divmod(c, 2)
        blocks = _perm_blocks(p)
        tok = np.concatenate([np.arange(b * P, (b + 1) * P) for b in blocks])
        own = np.concatenate([np.arange(b * P, (b + 1) * P) for b in blocks[0::2]])
        odd = np.concatenate([np.arange(b * P, (b + 1) * P) for b in blocks[1::2]])
        own_toks.append((s, own))
        in_maps.append({
            "xpe": np.ascontiguousarray(x[s][own]),
            "xpo": np.ascontiguousarray(x[s][odd].astype(ml_dtypes.bfloat16)),
            "qg": own.astype(np.float32),
            "kg": po(tok.astype(np.float32), NT),
            **shared,
        })
    return in_maps, own_toks


def kernel(x, ln1_scale, ln1_bias, Wqkv, bqkv, Wo, bo,
           ln2_scale, ln2_bias, Wfc, bfc, Wproj, bproj):
    from concourse.bass_utils import run_bass_kernel_spmd

    in_maps, own_toks = _prepare(x, ln1_scale, ln1_bias, Wqkv, bqkv, Wo, bo,
                                 ln2_scale, ln2_bias, Wfc, bfc, Wproj, bproj)
    nc = _get_nc()
    res = run_bass_kernel_spmd(nc, in_maps, list(range(N_CORES)))

    out = np.empty((B, T, C), np.float32)
    for c in range(N_CORES):
        s, own = own_toks[c]
        out[s][own] = res.results[c]["out"]
    return out
